# revision 1
# baseline (speedup 1.0000x reference)
"""Trainium2 Bass kernel for nn_EntailmentTransformerBlock — v2 (fused E-major).

Single fused pass per 256-token group, all activations E-major
([E_chunk=128 partitions, tokens]) end-to-end:
  - Inputs pre-transposed to E-major bf16 on the host (no PE transposes).
  - LayerNorm statistics via PE partition-reduction matmuls (ones/sel
    columns); rstd via DVE fast-inverse-sqrt (bit trick + 2 Newton steps)
    so ACT never loads the Sqrt table (Exp/Relu/Square share one table).
  - LN1 is folded into the FFN: W1 is pre-scaled by gamma1 on the host,
    the per-token rstd A commutes through ReLU (A>0) and is applied once
    after W2; the residual-stream gamma1*(x1-mu) enters the W2 PSUM via
    5 diagonal-block matmuls.
  - Per-token rows (mu, rstd, ...) are broadcast across partitions with
    rank-1 f32 matmuls (ones ⊗ row).
  - Software-pipelined emission: W1/W2 of group g interleaves with
    attention of group g+1 and input-proj of group g+2, keeping PE busy
    across the LayerNorm dependency chains.
Sharding: pure data-parallel over batch n (64) across 8 cores.
"""

import functools

import numpy as np
import ml_dtypes

import concourse.bass as bass
import concourse.tile as tile
from concourse import bacc, mybir
from concourse.bass_utils import run_bass_kernel_spmd

P = 128
E = 640
EC = 5           # E / 128 chunks
F = 2560
FC = 20          # F / 128 chunks
H = 5            # heads, head_dim = 128
NCORES = 8
NPAIRS = 16      # (n, s) pairs per core: 8 n * 2 s
GROUP = 2        # pairs per processing group
NG = NPAIRS // GROUP
TOKG = GROUP * P  # tokens per group = 256
EPS = 1e-5
SCALE = float(1.0 / np.sqrt(128.0))  # 1/sqrt(key_len)
INV_E = 1.0 / float(E)
MAGIC = 0x5F3759DF

f32 = mybir.dt.float32
bf16 = mybir.dt.bfloat16
i32 = mybir.dt.int32

AX = mybir.AxisListType.X
ALU = mybir.AluOpType
ACTF = mybir.ActivationFunctionType


def _rsqrt_row(nc, spool, ve, out, iters=2):
    """out = 1/sqrt(ve) elementwise on a [1, N] f32 row (DVE only).

    Fast-inverse-sqrt bit trick + Newton iterations (1 iter ~1.7e-3,
    2 iters ~5e-6 rel err).
    """
    n = ve.shape[-1]
    t = spool.tile([1, n], i32, tag="rsq_i")
    nc.vector.tensor_scalar(t, ve.bitcast(i32), 1, None,
                            op0=ALU.logical_shift_right)
    nc.vector.tensor_scalar(out.bitcast(i32), t, MAGIC, -1,
                            op0=ALU.subtract, op1=ALU.mult)
    for it in range(iters):
        # y <- y*(1.5 - 0.5*v*y^2), fused into 3 DVE ops
        t1 = spool.tile([1, n], f32, tag="rsq_n")
        nc.vector.tensor_tensor(t1, ve, out, op=ALU.mult)
        nc.vector.scalar_tensor_tensor(t1, t1, -0.5, out,
                                       op0=ALU.mult, op1=ALU.mult)
        nc.vector.scalar_tensor_tensor(out, t1, 1.5, out,
                                       op0=ALU.add, op1=ALU.mult)


def _emit(tc, io, has_t1, has_bsum, has_g2b, needs_max):
    nc = tc.nc
    from contextlib import ExitStack

    with ExitStack() as ctx:
        singles = ctx.enter_context(tc.tile_pool(name="singles", bufs=1))
        qin_pool = ctx.enter_context(tc.tile_pool(name="qin", bufs=3))
        kin_pool = ctx.enter_context(tc.tile_pool(name="kin", bufs=2))
        vin_pool = ctx.enter_context(tc.tile_pool(name="vin", bufs=2))
        madd_pool = ctx.enter_context(tc.tile_pool(name="madd", bufs=3))
        qkt_pool = ctx.enter_context(tc.tile_pool(name="qkt", bufs=2))
        vtok_pool = ctx.enter_context(tc.tile_pool(name="vtok", bufs=2))
        outt_pool = ctx.enter_context(tc.tile_pool(name="outt", bufs=2))
        attn_pool = ctx.enter_context(tc.tile_pool(name="attn", bufs=3))
        x1_pool = ctx.enter_context(tc.tile_pool(name="x1", bufs=2))
        h_pool = ctx.enter_context(tc.tile_pool(name="h", bufs=2))
        x2_pool = ctx.enter_context(tc.tile_pool(name="x2", bufs=2))
        out_pool = ctx.enter_context(tc.tile_pool(name="outk", bufs=2))
        # per-pair softmax statistics ([P, H] — tiny)
        hpool = ctx.enter_context(tc.tile_pool(name="hstats", bufs=4))
        # per-group [1, TOKG] rows and broadcast staging
        spool = ctx.enter_context(tc.tile_pool(name="rows", bufs=2))
        # large row tiles with within-iteration lifetime only
        spool1 = ctx.enter_context(tc.tile_pool(name="rows1", bufs=1))
        # PSUM: 4 x 2KB banks for 1-bank tiles, 2 x 2.5KB (2 banks) for
        # the 5-head attention tiles -> 8 banks total.
        psA = ctx.enter_context(tc.tile_pool(name="psA", bufs=4, space="PSUM"))
        psB = ctx.enter_context(tc.tile_pool(name="psB", bufs=2, space="PSUM"))

        # ---- weights / constants (resident) ----
        # DMA issue order matters: the HBM pipe is effectively serial, so
        # group-0 inputs must land right after wq/wk; w1g/w2 are deferred
        # (emitted after the prologue) since the FFN starts ~20us in.
        wq_sb = singles.tile([P, EC, E], bf16)
        nc.sync.dma_start(wq_sb, io["wq"].rearrange("(c p) o -> p c o", p=P))
        wk_sb = singles.tile([P, EC, E], bf16)
        wv_sb = singles.tile([P, EC, E], bf16)
        wo_sb = singles.tile([P, EC, E], bf16)
        w1_sb = singles.tile([P, EC, F], bf16)
        w2_sb = singles.tile([P, FC, E], bf16)
        dg1_sb = singles.tile([P, EC * P], bf16)
        vecs_sb = singles.tile([P, 4 * EC], f32)
        bo_sc = vecs_sb[:, 0:EC]
        bsum_sc = vecs_sb[:, EC:2 * EC]
        g2_sc = vecs_sb[:, 2 * EC:3 * EC]
        be2_sc = vecs_sb[:, 3 * EC:4 * EC]
        if has_t1:
            t1_sb = singles.tile([1, F], bf16)

        ones_f = singles.tile([1, P], f32)
        nc.vector.memset(ones_f, 1.0)
        ones_b = singles.tile([1, P], bf16)
        nc.vector.memset(ones_b, 1.0)
        # 1/E row: folds the mean division into the rank-1 broadcast matmul
        invE_b = singles.tile([1, P], bf16)
        nc.vector.memset(invE_b, INV_E)
        # ones column for M=1 partition reductions
        ones_col = singles.tile([P, 1], bf16)
        nc.vector.memset(ones_col, 1.0)

        ncopy = 0

        def pcopy(dst, src):
            nonlocal ncopy
            ncopy += 1
            if ncopy % 2:
                nc.vector.tensor_copy(dst, src)
            else:
                nc.scalar.copy(dst, src)

        def in_slice(name, g):
            return io[name][:, g * EC * TOKG:(g + 1) * EC * TOKG].rearrange(
                "p (c t) -> p c t", c=EC
            )

        # per-group state passed between pipeline stages
        S = [dict() for _ in range(NG)]

        def s1_dma(g, chunked=False):
            """Issue the group's input DMAs (hardware DGE queues only)."""
            st = S[g]
            tq = qin_pool.tile([P, EC, TOKG], bf16, tag="qin")
            tk = kin_pool.tile([P, EC, TOKG], bf16, tag="kin")
            if g == 0:
                # group 0 rides the scalar queue so it isn't serialized
                # behind wq on sync
                nc.scalar.dma_start(tq, in_slice("xqt", g))
            else:
                nc.sync.dma_start(tq, in_slice("xqt", g))
            nc.scalar.dma_start(tk, in_slice("xkt", g))
            tv = vin_pool.tile([P, EC, TOKG], bf16, tag="vin")
            nc.scalar.dma_start(tv, in_slice("xvt", g))
            maddt = madd_pool.tile([1, GROUP, P], bf16, tag="madd")
            nc.sync.dma_start(
                maddt,
                io["madd"][g * GROUP:(g + 1) * GROUP, :].rearrange(
                    "(o a) b -> o a b", o=1
                ),
            )
            st["xqT"], st["xkT"], st["xvT"], st["maddt"] = tq, tk, tv, maddt

        def qkproj(g, which):
            st = S[g]
            srcT = st["xqT"] if which == "q" else st["xkT"]
            w_sb = wq_sb if which == "q" else wk_sb
            dst = qkt_pool.tile([P, EC, TOKG], bf16, tag=f"{which}Tb")
            for eo in range(EC):
                ps = psA.tile([P, 2 * TOKG], f32, tag="a")
                for ci in range(EC):
                    nc.tensor.matmul(
                        ps[:, 0:TOKG],
                        lhsT=w_sb[:, ci, eo * P:(eo + 1) * P],
                        rhs=srcT[:, ci, :],
                        start=(ci == 0),
                        stop=(ci == EC - 1),
                    )
                pcopy(dst[:, eo, :], ps[:, 0:TOKG])
            st[f"{which}Tb"] = dst

        def vproj(g, pairs=(0, 1)):
            st = S[g]
            tv = st["xvT"]
            if 0 in pairs:
                v_tok = vtok_pool.tile([P, GROUP, E], bf16, tag="v_tok")
                st["v_tok"] = v_tok
            v_tok = st["v_tok"]
            for pr in pairs:
                ps = psB.tile([P, E], f32, tag="b")
                for n0, nsz in ((0, 512), (512, 128)):
                    for ci in range(EC):
                        nc.tensor.matmul(
                            ps[:, n0:n0 + nsz],
                            lhsT=tv[:, ci, pr * P:(pr + 1) * P],
                            rhs=wv_sb[:, ci, n0:n0 + nsz],
                            start=(ci == 0),
                            stop=(ci == EC - 1),
                        )
                pcopy(v_tok[:, pr, :], ps)

        def attn_energy(g, pr):
            """Energy -> masked softmax(axis=q) numerator + denominator."""
            st = S[g]
            if pr == 0:
                outT = outt_pool.tile([P, H, TOKG], bf16, tag="outT")
                st["outT"] = outT
            qTb, kTb, maddt = st["qTb"], st["kTb"], st["maddt"]
            tsl = slice(pr * P, (pr + 1) * P)
            pse5 = psB.tile([P, H, P], f32, tag="b")
            for h in range(H):
                nc.tensor.matmul(
                    pse5[:, h, :], lhsT=kTb[:, h, tsl], rhs=qTb[:, h, tsl],
                    start=True, stop=False,
                )
                nc.tensor.matmul(
                    pse5[:, h, :], lhsT=ones_b, rhs=maddt[:, pr, :],
                    start=False, stop=True,
                )
            attn5 = attn_pool.tile([P, H, P], bf16, tag="asb")
            ssum5 = hpool.tile([P, H], f32, tag="ssum5")
            if needs_max:
                # stabilized softmax (only needed if a row can be fully
                # masked: logits are otherwise O(5), well within exp range)
                mx5 = hpool.tile([P, H], f32, tag="mx5")
                nc.vector.reduce_max(out=mx5, in_=pse5, axis=AX)
                negb5 = hpool.tile([P, H], f32, tag="negb5")
                nc.vector.tensor_scalar_mul(negb5, mx5, -SCALE)
                for h in range(H):
                    nc.scalar.activation(
                        attn5[:, h, :], pse5[:, h, :], ACTF.Exp,
                        bias=negb5[:, h:h + 1], scale=SCALE,
                        accum_out=ssum5[:, h:h + 1],
                    )
            else:
                nc.scalar.activation(attn5, pse5, ACTF.Exp,
                                     bias=0.0, scale=SCALE)
                nc.vector.reduce_sum(out=ssum5, in_=attn5, axis=AX)
            rec5 = hpool.tile([P, H], f32, tag="rec5")
            nc.vector.reciprocal(rec5, ssum5)
            st[f"attn{pr}"], st[f"rec{pr}"] = attn5, rec5

        def attn_av(g, pr):
            """Normalize and apply attention to v for one pair."""
            st = S[g]
            tsl = slice(pr * P, (pr + 1) * P)
            attn5, rec5 = st[f"attn{pr}"], st[f"rec{pr}"]
            nc.vector.tensor_tensor(
                attn5, attn5, rec5[:, :, None].to_broadcast([P, H, P]),
                op=ALU.mult,
            )
            pso5 = psB.tile([P, H, P], f32, tag="b")
            for h in range(H):
                nc.tensor.matmul(
                    pso5[:, h, :],
                    lhsT=S[g]["v_tok"][:, pr, h * P:(h + 1) * P],
                    rhs=attn5[:, h, :],
                    start=True, stop=True,
                )
            # Wo gates on the pair-1 copy: keep that one on ACT's shorter
            # queue; pair 0 goes to DVE so ACT stays clear for the stat-row
            # copies that the next group's PSUM ring rotation waits on
            if pr == 1:
                nc.scalar.copy(st["outT"][:, :, tsl], pso5)
            else:
                nc.vector.tensor_copy(st["outT"][:, :, tsl], pso5)

        def wo_x1(g):
            """Wo projection + bo + residual -> x1 (bf16) + Pool squares."""
            st = S[g]
            outT = st["outT"]
            x1b = x1_pool.tile([P, EC, TOKG], bf16, tag="x1b")
            x1sq = x1_pool.tile([P, EC, TOKG], bf16, tag="x1sq")
            for eo in range(EC):
                ps = psA.tile([P, 2 * TOKG], f32, tag="a")
                for h in range(H):
                    nc.tensor.matmul(
                        ps[:, 0:TOKG],
                        lhsT=wo_sb[:, h, eo * P:(eo + 1) * P],
                        rhs=outT[:, h, :],
                        start=(h == 0),
                        stop=(h == H - 1),
                    )
                nc.vector.scalar_tensor_tensor(
                    x1b[:, eo, :], ps[:, 0:TOKG], bo_sc[:, eo:eo + 1],
                    st["xqT"][:, eo, :], op0=ALU.add, op1=ALU.add,
                )
                # square on the idle Pool engine (SBUF-only), per chunk
                nc.gpsimd.tensor_tensor(x1sq[:, eo, :], x1b[:, eo, :],
                                        x1b[:, eo, :], op=ALU.mult)
            st["x1b"], st["x1sq"] = x1b, x1sq

        def rowmath(g, s0r, s1r, out_rstd, iters):
            """rstd = 1/sqrt(sum1/E + eps - (sum0/E)^2) from raw-sum rows."""
            m2 = spool.tile([1, TOKG], f32, tag="m2")
            nc.vector.tensor_tensor(m2, s0r, s0r, op=ALU.mult)
            vep = spool.tile([1, TOKG], f32, tag="vep")
            nc.vector.tensor_scalar(vep, s1r, INV_E, EPS,
                                    op0=ALU.mult, op1=ALU.add)
            ve = spool.tile([1, TOKG], f32, tag="ve")
            nc.vector.scalar_tensor_tensor(ve, m2, -INV_E * INV_E, vep,
                                           op0=ALU.mult, op1=ALU.add)
            _rsqrt_row(nc, spool, ve, out_rstd, iters=iters)
            return ve

        def stats1(g):
            st = S[g]
            ps_st = psB.tile([P, E], f32, tag="b")
            for ci in range(EC):
                nc.tensor.matmul(
                    ps_st[0:1, 0:TOKG], lhsT=ones_col, rhs=st["x1b"][:, ci, :],
                    start=(ci == 0), stop=(ci == EC - 1),
                )
            # stage raw sums in SBUF via ACT (keeps DVE off the x1c chain);
            # bf16 suffices: mu is small and the LN1 rstd error cancels
            # through LN2's renormalization
            s0r = spool.tile([1, TOKG], bf16, tag="s0r")
            nc.scalar.copy(s0r, ps_st[0:1, 0:TOKG])
            st["s0r"] = s0r
            for ci in range(EC):
                nc.tensor.matmul(
                    ps_st[0:1, TOKG:2 * TOKG], lhsT=ones_col,
                    rhs=st["x1sq"][:, ci, :],
                    start=(ci == 0), stop=(ci == EC - 1),
                )
            s1r = spool.tile([1, TOKG], f32, tag="s1r")
            nc.scalar.copy(s1r, ps_st[0:1, TOKG:2 * TOKG])
            # rstd A: 1 Newton iter (its error cancels through LN2)
            A = spool.tile([1, TOKG], f32, tag="A")
            ve = rowmath(g, s0r, s1r, A, iters=1)
            st["A"] = A
            if has_t1 or has_bsum:
                sd = spool.tile([1, TOKG], bf16, tag="sd")
                nc.vector.tensor_tensor(sd, ve, A, op=ALU.mult)
                st["sd"] = sd

        def mu_b_x1c(g):
            st = S[g]
            ps_mu = psA.tile([P, 2 * TOKG], f32, tag="a")
            nc.tensor.matmul(ps_mu[:, 0:TOKG], lhsT=invE_b, rhs=st["s0r"],
                             start=True, stop=True)
            x1c = x1_pool.tile([P, EC, TOKG], bf16, tag="x1c")
            for ci in range(EC):
                nc.vector.tensor_tensor(
                    x1c[:, ci, :], st["x1b"][:, ci, :], ps_mu[:, 0:TOKG],
                    op=ALU.subtract,
                )
            st["x1c"] = x1c

        def ffn_w1(g):
            st = S[g]
            x1c = st["x1c"]
            hp = h_pool.tile([P, FC, TOKG], bf16, tag="hp")
            for f in range(FC):
                ps = psA.tile([P, 2 * TOKG], f32, tag="a")
                for ci in range(EC):
                    nc.tensor.matmul(
                        ps[:, 0:TOKG],
                        lhsT=w1_sb[:, ci, f * P:(f + 1) * P],
                        rhs=x1c[:, ci, :],
                        start=(ci == 0),
                        stop=(ci == EC - 1) and not has_t1,
                    )
                if has_t1:
                    nc.tensor.matmul(
                        ps[:, 0:TOKG],
                        lhsT=t1_sb[0:1, f * P:(f + 1) * P],
                        rhs=st["sd"],
                        start=False, stop=True,
                    )
                nc.scalar.activation(hp[:, f, :], ps[:, 0:TOKG], ACTF.Relu,
                                     bias=0.0, scale=1.0)
            st["hp"] = hp

        def ffn_w2(g, interleave_stats=False):
            """W2 + diag(g1) residual. For the last group (no later work to
            hide the LN2 chain behind) the stat reductions interleave into
            the chunk stream, using two separate PSUM banks — two
            accumulation groups must never share a bank interleaved."""
            st = S[g]
            x1c, hp = st["x1c"], st["hp"]
            # broadcast A over partitions; stage in SBUF for the DVE mults
            ps_A = psA.tile([P, 2 * TOKG], f32, tag="a")
            nc.tensor.matmul(ps_A[:, 0:TOKG], lhsT=ones_f, rhs=st["A"],
                             start=True, stop=True)
            A_bs = spool.tile([P, TOKG], f32, tag="A_bs")
            pcopy(A_bs, ps_A[:, 0:TOKG])

            x2b = x2_pool.tile([P, EC, TOKG], bf16, tag="x2b")
            x2sq = x2_pool.tile([P, EC, TOKG], bf16, tag="x2sq")
            if interleave_stats:
                ps_sum = psB.tile([P, E], f32, tag="b")
                ps_sq = psB.tile([P, E], f32, tag="b")

            for eo in range(EC):
                ps = psA.tile([P, 2 * TOKG], f32, tag="a")
                for f in range(FC):
                    nc.tensor.matmul(
                        ps[:, 0:TOKG],
                        lhsT=w2_sb[:, f, eo * P:(eo + 1) * P],
                        rhs=hp[:, f, :],
                        start=(f == 0), stop=False,
                    )
                nc.tensor.matmul(
                    ps[:, 0:TOKG],
                    lhsT=dg1_sb[:, eo * P:(eo + 1) * P],
                    rhs=x1c[:, eo, :],
                    start=False, stop=not has_bsum,
                )
                if has_bsum:
                    nc.tensor.matmul(
                        ps[:, 0:TOKG],
                        lhsT=bsr_sb[0:1, eo * P:(eo + 1) * P],
                        rhs=st["sd"],
                        start=False, stop=True,
                    )
                nc.vector.tensor_tensor(x2b[:, eo, :], ps[:, 0:TOKG], A_bs,
                                        op=ALU.mult)
                if interleave_stats:
                    # last group: the sq -> stats -> rowmath chain is the
                    # exposed tail; ACT's square is ~2x faster than Pool's
                    nc.scalar.activation(x2sq[:, eo, :], x2b[:, eo, :],
                                         ACTF.Square, bias=0.0, scale=1.0)
                else:
                    nc.gpsimd.tensor_tensor(x2sq[:, eo, :], x2b[:, eo, :],
                                            x2b[:, eo, :], op=ALU.mult)
                if interleave_stats:
                    if eo >= 1:
                        nc.tensor.matmul(
                            ps_sum[0:1, 0:TOKG], lhsT=ones_col,
                            rhs=x2b[:, eo - 1, :],
                            start=(eo == 1), stop=(eo == EC - 1) and False,
                        )
                    if eo >= 2:
                        nc.tensor.matmul(
                            ps_sq[0:1, 0:TOKG], lhsT=ones_col,
                            rhs=x2sq[:, eo - 2, :],
                            start=(eo == 2), stop=False,
                        )
            if interleave_stats:
                nc.tensor.matmul(ps_sum[0:1, 0:TOKG], lhsT=ones_col,
                                 rhs=x2b[:, EC - 1, :], start=False, stop=True)
                nc.tensor.matmul(ps_sq[0:1, 0:TOKG], lhsT=ones_col,
                                 rhs=x2sq[:, EC - 2, :], start=False,
                                 stop=False)
                nc.tensor.matmul(ps_sq[0:1, 0:TOKG], lhsT=ones_col,
                                 rhs=x2sq[:, EC - 1, :], start=False,
                                 stop=True)
                s0r = spool.tile([1, TOKG], bf16, tag="s0r2")
                nc.scalar.copy(s0r, ps_sum[0:1, 0:TOKG])
                s1r = spool.tile([1, TOKG], f32, tag="s1r2")
                nc.scalar.copy(s1r, ps_sq[0:1, 0:TOKG])
                st["s0r2"], st["s1r2"] = s0r, s1r
            st["x2b"], st["x2sq"] = x2b, x2sq

        def stats2(g):
            st = S[g]
            if "s0r2" not in st:
                ps_st = psB.tile([P, E], f32, tag="b")
                for ci in range(EC):
                    nc.tensor.matmul(
                        ps_st[0:1, 0:TOKG], lhsT=ones_col,
                        rhs=st["x2b"][:, ci, :],
                        start=(ci == 0), stop=(ci == EC - 1),
                    )
                for ci in range(EC):
                    nc.tensor.matmul(
                        ps_st[0:1, TOKG:2 * TOKG], lhsT=ones_col,
                        rhs=st["x2sq"][:, ci, :],
                        start=(ci == 0), stop=(ci == EC - 1),
                    )
                s0r = spool.tile([1, TOKG], bf16, tag="s0r2")
                nc.scalar.copy(s0r, ps_st[0:1, 0:TOKG])
                s1r = spool.tile([1, TOKG], f32, tag="s1r2")
                nc.scalar.copy(s1r, ps_st[0:1, TOKG:2 * TOKG])
                st["s0r2"], st["s1r2"] = s0r, s1r
            # A2 row f32 (applied directly to the output: full precision);
            # C2 shift row bf16 (|C2| ~ |mu2*A2| is small — rounding is
            # negligible, and the bf16 rank-1 broadcast is 4x cheaper)
            A2 = spool1.tile([1, TOKG], f32, tag="bc2")
            # last group: its rowmath chain is fully exposed (no later PE
            # work); 1 Newton iter costs 0.17% rstd error on 1/8 of tokens
            # (~6e-4 globally) against a 4.6x error margin
            rowmath(g, st["s0r2"], st["s1r2"], A2,
                    iters=1 if g == NG - 1 else 2)
            c2 = spool.tile([1, TOKG], bf16, tag="c2")
            nc.vector.scalar_tensor_tensor(c2, st["s0r2"], -INV_E, A2,
                                           op0=ALU.mult, op1=ALU.mult)
            st["A2"], st["c2"] = A2, c2

        def tail_final(g, split=False):
            st = S[g]
            ps_bc = psB.tile([P, E], f32, tag="b")
            nc.tensor.matmul(ps_bc[:, 0:TOKG], lhsT=ones_f, rhs=st["A2"],
                             start=True, stop=True)
            nc.tensor.matmul(ps_bc[:, TOKG:2 * TOKG], lhsT=ones_b,
                             rhs=st["c2"], start=True, stop=True)
            if split:
                # last group: no later PE work hides this chain, so stage
                # the broadcasts in SBUF and fan the chunks out over
                # DVE and Pool in parallel
                bc_s = spool1.tile([P, 2 * TOKG], f32, tag="bc_s")
                nc.scalar.copy(bc_s, ps_bc[:, 0:2 * TOKG])
            ot = out_pool.tile([P, EC, TOKG], bf16, tag="outk")
            for c in range(EC):
                if split and c >= 3:
                    eng, a2s, c2s = (nc.gpsimd, bc_s[:, 0:TOKG],
                                     bc_s[:, TOKG:2 * TOKG])
                else:
                    eng, a2s, c2s = (nc.vector, ps_bc[:, 0:TOKG],
                                     ps_bc[:, TOKG:2 * TOKG])
                eng.tensor_tensor(
                    ot[:, c, :], st["x2b"][:, c, :], a2s, op=ALU.mult,
                )
                eng.tensor_tensor(
                    ot[:, c, :], ot[:, c, :], c2s, op=ALU.add,
                )
                if has_g2b:
                    eng.tensor_scalar(
                        ot[:, c, :], ot[:, c, :], g2_sc[:, c:c + 1],
                        be2_sc[:, c:c + 1], op0=ALU.mult, op1=ALU.add,
                    )
                oq = nc.scalar if (split and c % 2) else nc.sync
                oq.dma_start(
                    io["out"][
                        :,
                        (g * EC + c) * TOKG:(g * EC + c + 1) * TOKG
                    ],
                    ot[:, c, :],
                )

        if has_bsum:
            bsr_sb = singles.tile([1, E], bf16)

        # ---- software-pipelined schedule ----
        # Weave group g's FFN with group g+1's attention and group g+2's
        # projections so PE never drains on the LayerNorm latency chains.
        s1_dma(0)
        s1_dma(1)
        # Issue-delay gadget: the DMA engine serializes all queues, and
        # Pool would otherwise enqueue wk/wv at t=0, beating group-0's
        # inputs in the round-robin. A tiny Pool op gated on the first
        # madd DMA holds their issue until the critical inputs are in
        # flight; wk still lands before the k-projection needs it.
        dly = spool.tile([1, GROUP, P], bf16, tag="dly")
        nc.gpsimd.tensor_copy(dly, S[0]["maddt"])
        nc.gpsimd.dma_start(wk_sb, io["wk"].rearrange("(c p) o -> p c o", p=P))
        nc.gpsimd.dma_start(wv_sb, io["wv"].rearrange("(c p) o -> p c o", p=P))
        nc.gpsimd.dma_start(wo_sb, io["wo"].rearrange("(c p) o -> p c o", p=P))
        nc.gpsimd.dma_start(dg1_sb, io["dg1"])
        nc.gpsimd.dma_start(vecs_sb, io["vecs"])
        if has_t1:
            nc.gpsimd.dma_start(t1_sb, io["t1"])
        if has_bsum:
            nc.gpsimd.dma_start(bsr_sb, io["bsr"])
        nc.sync.dma_start(w1_sb, io["w1g"].rearrange("(c p) o -> p c o", p=P))
        qkproj(0, "q")
        qkproj(0, "k")
        nc.sync.dma_start(w2_sb, io["w2"].rearrange("(c p) o -> p c o", p=P))
        vproj(0)
        attn_energy(0, 0)
        qkproj(1, "q")
        attn_av(0, 0)
        attn_energy(0, 1)
        qkproj(1, "k")
        attn_av(0, 1)
        wo_x1(0)
        stats1(0)
        mu_b_x1c(0)
        vproj(1)
        for g in range(NG):
            if g + 2 < NG:
                s1_dma(g + 2)
            ffn_w1(g)
            ffn_w2(g, interleave_stats=(g == NG - 1))
            if g + 1 < NG:
                attn_energy(g + 1, 0)
                if g + 2 < NG:
                    qkproj(g + 2, "q")
                attn_av(g + 1, 0)
                attn_energy(g + 1, 1)
                if g + 2 < NG:
                    qkproj(g + 2, "k")
                stats2(g)
                if g + 2 < NG:
                    vproj(g + 2, pairs=(0,))
                attn_av(g + 1, 1)
                wo_x1(g + 1)
                stats1(g + 1)
                mu_b_x1c(g + 1)
                if g + 2 < NG:
                    vproj(g + 2, pairs=(1,))
                tail_final(g)
            else:
                stats2(g)
                tail_final(g, split=True)
            S[g] = None


@functools.lru_cache(maxsize=4)
def _build(has_t1=False, has_bsum=False, has_g2b=False, needs_max=False):
    nc = bacc.Bacc(
        "TRN2", target_bir_lowering=False, debug=False, num_devices=NCORES
    )
    ntok = NPAIRS * P
    io = {
        "xqt": nc.dram_tensor("xqt", [P, EC * ntok], bf16, kind="ExternalInput").ap(),
        "xkt": nc.dram_tensor("xkt", [P, EC * ntok], bf16, kind="ExternalInput").ap(),
        "xvt": nc.dram_tensor("xvt", [P, EC * ntok], bf16, kind="ExternalInput").ap(),
        "madd": nc.dram_tensor("madd", [NPAIRS, P], bf16, kind="ExternalInput").ap(),
        "wq": nc.dram_tensor("wq", [E, E], bf16, kind="ExternalInput").ap(),
        "wk": nc.dram_tensor("wk", [E, E], bf16, kind="ExternalInput").ap(),
        "wv": nc.dram_tensor("wv", [E, E], bf16, kind="ExternalInput").ap(),
        "wo": nc.dram_tensor("wo", [E, E], bf16, kind="ExternalInput").ap(),
        "w1g": nc.dram_tensor("w1g", [E, F], bf16, kind="ExternalInput").ap(),
        "w2": nc.dram_tensor("w2", [F, E], bf16, kind="ExternalInput").ap(),
        "dg1": nc.dram_tensor("dg1", [P, EC * P], bf16, kind="ExternalInput").ap(),
        "vecs": nc.dram_tensor("vecs", [P, 4 * EC], f32, kind="ExternalInput").ap(),
        "out": nc.dram_tensor("out", [P, EC * ntok], bf16, kind="ExternalOutput").ap(),
    }
    if has_t1:
        io["t1"] = nc.dram_tensor("t1", [1, F], f32, kind="ExternalInput").ap()
    if has_bsum:
        io["bsr"] = nc.dram_tensor("bsr", [1, E], f32, kind="ExternalInput").ap()
    with tile.TileContext(nc) as tc:
        _emit(tc, io, has_t1, has_bsum, has_g2b, needs_max)
    nc.compile()
    return nc


def _prep(value, key, query, mask, Wv, Wk, Wq, Wo, bo, W1, b1, W2, b2,
          g1, be1, g2, be2):
    """Returns (flags, shared_map, per_core_maps)."""
    bfl = ml_dtypes.bfloat16
    f32n = np.float32
    g1 = np.asarray(g1, f32n)
    t1 = np.asarray(be1, f32n) @ np.asarray(W1, f32n) + np.asarray(b1, f32n)
    bsum = np.asarray(be1, f32n) + np.asarray(b2, f32n)
    has_t1 = bool(np.any(t1 != 0))
    has_bsum = bool(np.any(bsum != 0))
    has_g2b = bool(np.any(np.asarray(g2, f32n) != 1.0)
                   or np.any(np.asarray(be2, f32n) != 0.0))
    # stabilized softmax only needed if some query row is fully masked
    # (otherwise logits are O(5) and raw exp is safe)
    needs_max = bool(np.any(np.all(np.asarray(mask)[:, :, :, 0] == 0, axis=2)))

    w1g = (np.asarray(W1, f32n) * g1[:, None]).astype(bfl)
    dg1 = np.zeros((P, EC * P), f32n)
    idx = np.arange(P)
    for c in range(EC):
        dg1[idx, c * P + idx] = g1[c * P + idx]

    def cols(v):
        return np.asarray(v, f32n).reshape(EC, P).T  # [P, EC]

    vecs = np.concatenate(
        [cols(bo), cols(bsum), cols(g2), cols(be2)], axis=1
    ).astype(f32n)

    shared = {
        "wq": np.ascontiguousarray(np.asarray(Wq, f32n).astype(bfl)),
        "wk": np.ascontiguousarray(np.asarray(Wk, f32n).astype(bfl)),
        "wv": np.ascontiguousarray(np.asarray(Wv, f32n).astype(bfl)),
        "wo": np.ascontiguousarray(np.asarray(Wo, f32n).astype(bfl)),
        "w1g": np.ascontiguousarray(w1g),
        "w2": np.ascontiguousarray(np.asarray(W2, f32n).astype(bfl)),
        "dg1": np.ascontiguousarray(dg1.astype(bfl)),
        "vecs": np.ascontiguousarray(vecs),
    }
    if has_t1:
        shared["t1"] = np.ascontiguousarray(t1.reshape(1, F).astype(bfl))
    if has_bsum:
        shared["bsr"] = np.ascontiguousarray(bsum.reshape(1, E).astype(bfl))

    def emaj(x2d):
        # [ntok, E] f32 -> [P, NG, EC, TOKG] bf16 -> flat [P, EC*ntok]
        a = np.asarray(x2d, f32n).reshape(NG, TOKG, EC, P)
        return np.ascontiguousarray(
            a.transpose(3, 0, 2, 1).astype(bfl).reshape(P, EC * NG * TOKG)
        )

    npc = 64 // NCORES
    ntok = NPAIRS * P
    in_maps = []
    for c in range(NCORES):
        nsl = slice(c * npc, (c + 1) * npc)
        madd = np.where(
            np.asarray(mask)[nsl, :, :, 0] == 0, f32n(-1e20), f32n(0.0)
        ).reshape(NPAIRS, P).astype(bfl)
        in_maps.append(
            {
                "xqt": emaj(np.asarray(query)[nsl].reshape(ntok, E)),
                "xkt": emaj(np.asarray(key)[nsl].reshape(ntok, E)),
                "xvt": emaj(np.asarray(value)[nsl].reshape(ntok, E)),
                "madd": np.ascontiguousarray(madd),
                **shared,
            }
        )
    return (has_t1, has_bsum, has_g2b, needs_max), in_maps


def _prep_in_maps(**inputs):
    flags, in_maps = _prep(**{k: np.asarray(v) for k, v in inputs.items()})
    return in_maps


def _gather(res):
    outs = []
    for r in res.results:
        a = r["out"].reshape(P, NG, EC, TOKG)
        outs.append(a.transpose(1, 3, 2, 0).reshape(NPAIRS * P, E))
    out = np.concatenate(outs, axis=0)
    return out.reshape(64, 2, P, E).astype(np.float32)


def gather_concat(out_concat):
    """Reassemble a [NCORES*P, EC*ntok] concatenated raw output (as produced
    by per-core sharding along axis 0) into the full [64, 2, P, E] result."""
    a = np.asarray(out_concat).reshape(NCORES, P, NG, EC, TOKG)
    a = a.transpose(0, 2, 4, 3, 1).reshape(NCORES * NPAIRS * P, E)
    return a.reshape(64, 2, P, E).astype(np.float32)


def kernel(**inputs) -> np.ndarray:
    flags, in_maps = _prep(**{k: np.asarray(v) for k, v in inputs.items()})
    nc = _build(*flags)
    res = run_bass_kernel_spmd(nc, in_maps, core_ids=list(range(NCORES)))
    return _gather(res)


def run_traced(**inputs):
    flags, in_maps = _prep(**{k: np.asarray(v) for k, v in inputs.items()})
    nc = _build(*flags)
    res = run_bass_kernel_spmd(
        nc, in_maps, core_ids=list(range(NCORES)), trace=True
    )
    return _gather(res), res



# revision 4
# speedup vs baseline: 196.5930x; 196.5930x over previous
"""Trainium2 Bass kernel for nn_EntailmentTransformerBlock — v2 (fused E-major).

Single fused pass per 256-token group, all activations E-major
([E_chunk=128 partitions, tokens]) end-to-end:
  - Inputs pre-transposed to E-major bf16 on the host (no PE transposes).
  - LayerNorm statistics via PE partition-reduction matmuls (ones/sel
    columns); rstd via DVE fast-inverse-sqrt (bit trick + 2 Newton steps)
    so ACT never loads the Sqrt table (Exp/Relu/Square share one table).
  - LN1 is folded into the FFN: W1 is pre-scaled by gamma1 on the host,
    the per-token rstd A commutes through ReLU (A>0) and is applied once
    after W2; the residual-stream gamma1*(x1-mu) enters the W2 PSUM via
    5 diagonal-block matmuls.
  - Per-token rows (mu, rstd, ...) are broadcast across partitions with
    rank-1 f32 matmuls (ones ⊗ row).
  - Software-pipelined emission: W1/W2 of group g interleaves with
    attention of group g+1 and input-proj of group g+2, keeping PE busy
    across the LayerNorm dependency chains.
Sharding: pure data-parallel over batch n (64) across 8 cores.
"""

import functools

import numpy as np
import ml_dtypes

import concourse.bass as bass
import concourse.tile as tile
from concourse import bacc, mybir
from concourse.bass_utils import run_bass_kernel_spmd

P = 128
E = 640
EC = 5           # E / 128 chunks
F = 2560
FC = 20          # F / 128 chunks
H = 5            # heads, head_dim = 128
NCORES = 8
NPAIRS = 16      # (n, s) pairs per core: 8 n * 2 s
GROUP = 2        # pairs per processing group
NG = NPAIRS // GROUP
TOKG = GROUP * P  # tokens per group = 256
EPS = 1e-5
SCALE = float(1.0 / np.sqrt(128.0))  # 1/sqrt(key_len)
INV_E = 1.0 / float(E)
MAGIC = 0x5F3759DF

f32 = mybir.dt.float32
bf16 = mybir.dt.bfloat16
i32 = mybir.dt.int32

AX = mybir.AxisListType.X
ALU = mybir.AluOpType
ACTF = mybir.ActivationFunctionType


def _rsqrt_row(nc, spool, ve, out, iters=2):
    """out = 1/sqrt(ve) elementwise on a [1, N] f32 row (DVE only).

    Fast-inverse-sqrt bit trick + Newton iterations (1 iter ~1.7e-3,
    2 iters ~5e-6 rel err).
    """
    n = ve.shape[-1]
    t = spool.tile([1, n], i32, tag="rsq_i")
    nc.vector.tensor_scalar(t, ve.bitcast(i32), 1, None,
                            op0=ALU.logical_shift_right)
    nc.vector.tensor_scalar(out.bitcast(i32), t, MAGIC, -1,
                            op0=ALU.subtract, op1=ALU.mult)
    for it in range(iters):
        # y <- y*(1.5 - 0.5*v*y^2), fused into 3 DVE ops
        t1 = spool.tile([1, n], f32, tag="rsq_n")
        nc.vector.tensor_tensor(t1, ve, out, op=ALU.mult)
        nc.vector.scalar_tensor_tensor(t1, t1, -0.5, out,
                                       op0=ALU.mult, op1=ALU.mult)
        nc.vector.scalar_tensor_tensor(out, t1, 1.5, out,
                                       op0=ALU.add, op1=ALU.mult)


def _emit(tc, io, has_t1, has_bsum, has_g2b, needs_max):
    nc = tc.nc
    from contextlib import ExitStack

    with ExitStack() as ctx:
        singles = ctx.enter_context(tc.tile_pool(name="singles", bufs=1))
        qin_pool = ctx.enter_context(tc.tile_pool(name="qin", bufs=3))
        kin_pool = ctx.enter_context(tc.tile_pool(name="kin", bufs=2))
        vin_pool = ctx.enter_context(tc.tile_pool(name="vin", bufs=2))
        madd_pool = ctx.enter_context(tc.tile_pool(name="madd", bufs=3))
        qkt_pool = ctx.enter_context(tc.tile_pool(name="qkt", bufs=2))
        vtok_pool = ctx.enter_context(tc.tile_pool(name="vtok", bufs=2))
        outt_pool = ctx.enter_context(tc.tile_pool(name="outt", bufs=2))
        attn_pool = ctx.enter_context(tc.tile_pool(name="attn", bufs=3))
        x1_pool = ctx.enter_context(tc.tile_pool(name="x1", bufs=2))
        h_pool = ctx.enter_context(tc.tile_pool(name="h", bufs=2))
        x2_pool = ctx.enter_context(tc.tile_pool(name="x2", bufs=2))
        out_pool = ctx.enter_context(tc.tile_pool(name="outk", bufs=2))
        # per-pair softmax statistics ([P, H] — tiny)
        hpool = ctx.enter_context(tc.tile_pool(name="hstats", bufs=4))
        # per-group [1, TOKG] rows and broadcast staging
        spool = ctx.enter_context(tc.tile_pool(name="rows", bufs=2))
        # large row tiles with within-iteration lifetime only
        spool1 = ctx.enter_context(tc.tile_pool(name="rows1", bufs=1))
        # PSUM: 4 x 2KB banks for 1-bank tiles, 2 x 2.5KB (2 banks) for
        # the 5-head attention tiles -> 8 banks total.
        psA = ctx.enter_context(tc.tile_pool(name="psA", bufs=4, space="PSUM"))
        psB = ctx.enter_context(tc.tile_pool(name="psB", bufs=2, space="PSUM"))

        # ---- weights / constants (resident) ----
        # DMA issue order matters: the HBM pipe is effectively serial, so
        # group-0 inputs must land right after wq/wk; w1g/w2 are deferred
        # (emitted after the prologue) since the FFN starts ~20us in.
        wq_sb = singles.tile([P, EC, E], bf16)
        nc.sync.dma_start(wq_sb, io["wq"].rearrange("(c p) o -> p c o", p=P))
        wk_sb = singles.tile([P, EC, E], bf16)
        wv_sb = singles.tile([P, EC, E], bf16)
        wo_sb = singles.tile([P, EC, E], bf16)
        w1_sb = singles.tile([P, EC, F], bf16)
        w2_sb = singles.tile([P, FC, E], bf16)
        dg1_sb = singles.tile([P, EC * P], bf16)
        vecs_sb = singles.tile([P, 4 * EC], f32)
        bo_sc = vecs_sb[:, 0:EC]
        bsum_sc = vecs_sb[:, EC:2 * EC]
        g2_sc = vecs_sb[:, 2 * EC:3 * EC]
        be2_sc = vecs_sb[:, 3 * EC:4 * EC]
        if has_t1:
            t1_sb = singles.tile([1, F], bf16)

        ones_f = singles.tile([1, P], f32)
        nc.vector.memset(ones_f, 1.0)
        ones_b = singles.tile([1, P], bf16)
        nc.vector.memset(ones_b, 1.0)
        # 1/E row: folds the mean division into the rank-1 broadcast matmul
        invE_b = singles.tile([1, P], bf16)
        nc.vector.memset(invE_b, INV_E)
        # ones column for M=1 partition reductions
        ones_col = singles.tile([P, 1], bf16)
        nc.vector.memset(ones_col, 1.0)

        ncopy = 0

        def pcopy(dst, src):
            nonlocal ncopy
            ncopy += 1
            if ncopy % 2:
                nc.vector.tensor_copy(dst, src)
            else:
                nc.scalar.copy(dst, src)

        def in_slice(name, g):
            return io[name][:, g * EC * TOKG:(g + 1) * EC * TOKG].rearrange(
                "p (c t) -> p c t", c=EC
            )

        # per-group state passed between pipeline stages
        S = [dict() for _ in range(NG)]

        def s1_dma(g, chunked=False):
            """Issue the group's input DMAs (hardware DGE queues only)."""
            st = S[g]
            tq = qin_pool.tile([P, EC, TOKG], bf16, tag="qin")
            tk = kin_pool.tile([P, EC, TOKG], bf16, tag="kin")
            if g == 0:
                # group 0 rides the scalar queue so it isn't serialized
                # behind wq on sync
                nc.scalar.dma_start(tq, in_slice("xqt", g))
            else:
                nc.sync.dma_start(tq, in_slice("xqt", g))
            nc.scalar.dma_start(tk, in_slice("xkt", g))
            tv = vin_pool.tile([P, EC, TOKG], bf16, tag="vin")
            nc.scalar.dma_start(tv, in_slice("xvt", g))
            maddt = madd_pool.tile([1, GROUP, P], bf16, tag="madd")
            nc.sync.dma_start(
                maddt,
                io["madd"][g * GROUP:(g + 1) * GROUP, :].rearrange(
                    "(o a) b -> o a b", o=1
                ),
            )
            st["xqT"], st["xkT"], st["xvT"], st["maddt"] = tq, tk, tv, maddt

        def qkproj(g, which):
            st = S[g]
            srcT = st["xqT"] if which == "q" else st["xkT"]
            w_sb = wq_sb if which == "q" else wk_sb
            dst = qkt_pool.tile([P, EC, TOKG], bf16, tag=f"{which}Tb")
            for eo in range(EC):
                ps = psA.tile([P, 2 * TOKG], f32, tag="a")
                for ci in range(EC):
                    nc.tensor.matmul(
                        ps[:, 0:TOKG],
                        lhsT=w_sb[:, ci, eo * P:(eo + 1) * P],
                        rhs=srcT[:, ci, :],
                        start=(ci == 0),
                        stop=(ci == EC - 1),
                    )
                pcopy(dst[:, eo, :], ps[:, 0:TOKG])
            st[f"{which}Tb"] = dst

        def vproj(g, pairs=(0, 1)):
            st = S[g]
            tv = st["xvT"]
            if 0 in pairs:
                v_tok = vtok_pool.tile([P, GROUP, E], bf16, tag="v_tok")
                st["v_tok"] = v_tok
            v_tok = st["v_tok"]
            for pr in pairs:
                ps = psB.tile([P, E], f32, tag="b")
                for n0, nsz in ((0, 512), (512, 128)):
                    for ci in range(EC):
                        nc.tensor.matmul(
                            ps[:, n0:n0 + nsz],
                            lhsT=tv[:, ci, pr * P:(pr + 1) * P],
                            rhs=wv_sb[:, ci, n0:n0 + nsz],
                            start=(ci == 0),
                            stop=(ci == EC - 1),
                        )
                pcopy(v_tok[:, pr, :], ps)

        def attn_energy(g, pr):
            """Energy -> masked softmax(axis=q) numerator + denominator."""
            st = S[g]
            if pr == 0:
                outT = outt_pool.tile([P, H, TOKG], bf16, tag="outT")
                st["outT"] = outT
            qTb, kTb, maddt = st["qTb"], st["kTb"], st["maddt"]
            tsl = slice(pr * P, (pr + 1) * P)
            pse5 = psB.tile([P, H, P], f32, tag="b")
            for h in range(H):
                nc.tensor.matmul(
                    pse5[:, h, :], lhsT=kTb[:, h, tsl], rhs=qTb[:, h, tsl],
                    start=True, stop=False,
                )
                nc.tensor.matmul(
                    pse5[:, h, :], lhsT=ones_b, rhs=maddt[:, pr, :],
                    start=False, stop=True,
                )
            attn5 = attn_pool.tile([P, H, P], bf16, tag="asb")
            ssum5 = hpool.tile([P, H], f32, tag="ssum5")
            if needs_max:
                # stabilized softmax (only needed if a row can be fully
                # masked: logits are otherwise O(5), well within exp range)
                mx5 = hpool.tile([P, H], f32, tag="mx5")
                nc.vector.reduce_max(out=mx5, in_=pse5, axis=AX)
                negb5 = hpool.tile([P, H], f32, tag="negb5")
                nc.vector.tensor_scalar_mul(negb5, mx5, -SCALE)
                for h in range(H):
                    nc.scalar.activation(
                        attn5[:, h, :], pse5[:, h, :], ACTF.Exp,
                        bias=negb5[:, h:h + 1], scale=SCALE,
                        accum_out=ssum5[:, h:h + 1],
                    )
            else:
                nc.scalar.activation(attn5, pse5, ACTF.Exp,
                                     bias=0.0, scale=SCALE)
                nc.vector.reduce_sum(out=ssum5, in_=attn5, axis=AX)
            rec5 = hpool.tile([P, H], f32, tag="rec5")
            nc.vector.reciprocal(rec5, ssum5)
            st[f"attn{pr}"], st[f"rec{pr}"] = attn5, rec5

        def attn_av(g, pr):
            """Normalize and apply attention to v for one pair."""
            st = S[g]
            tsl = slice(pr * P, (pr + 1) * P)
            attn5, rec5 = st[f"attn{pr}"], st[f"rec{pr}"]
            nc.vector.tensor_tensor(
                attn5, attn5, rec5[:, :, None].to_broadcast([P, H, P]),
                op=ALU.mult,
            )
            pso5 = psB.tile([P, H, P], f32, tag="b")
            for h in range(H):
                nc.tensor.matmul(
                    pso5[:, h, :],
                    lhsT=S[g]["v_tok"][:, pr, h * P:(h + 1) * P],
                    rhs=attn5[:, h, :],
                    start=True, stop=True,
                )
            # Wo gates on the pair-1 copy: keep that one on ACT's shorter
            # queue; pair 0 goes to DVE so ACT stays clear for the stat-row
            # copies that the next group's PSUM ring rotation waits on
            if pr == 1:
                nc.scalar.copy(st["outT"][:, :, tsl], pso5)
            else:
                nc.vector.tensor_copy(st["outT"][:, :, tsl], pso5)

        def wo_x1(g):
            """Wo projection + bo + residual -> x1 (bf16) + Pool squares."""
            st = S[g]
            outT = st["outT"]
            x1b = x1_pool.tile([P, EC, TOKG], bf16, tag="x1b")
            x1sq = x1_pool.tile([P, EC, TOKG], bf16, tag="x1sq")
            for eo in range(EC):
                ps = psA.tile([P, 2 * TOKG], f32, tag="a")
                for h in range(H):
                    nc.tensor.matmul(
                        ps[:, 0:TOKG],
                        lhsT=wo_sb[:, h, eo * P:(eo + 1) * P],
                        rhs=outT[:, h, :],
                        start=(h == 0),
                        stop=(h == H - 1),
                    )
                nc.vector.scalar_tensor_tensor(
                    x1b[:, eo, :], ps[:, 0:TOKG], bo_sc[:, eo:eo + 1],
                    st["xqT"][:, eo, :], op0=ALU.add, op1=ALU.add,
                )
                # square on the idle Pool engine (SBUF-only), per chunk
                nc.gpsimd.tensor_tensor(x1sq[:, eo, :], x1b[:, eo, :],
                                        x1b[:, eo, :], op=ALU.mult)
            st["x1b"], st["x1sq"] = x1b, x1sq

        def rowmath(g, s0r, s1r, out_rstd, iters):
            """rstd = 1/sqrt(sum1/E + eps - (sum0/E)^2) from raw-sum rows."""
            m2 = spool.tile([1, TOKG], f32, tag="m2")
            nc.vector.tensor_tensor(m2, s0r, s0r, op=ALU.mult)
            vep = spool.tile([1, TOKG], f32, tag="vep")
            nc.vector.tensor_scalar(vep, s1r, INV_E, EPS,
                                    op0=ALU.mult, op1=ALU.add)
            ve = spool.tile([1, TOKG], f32, tag="ve")
            nc.vector.scalar_tensor_tensor(ve, m2, -INV_E * INV_E, vep,
                                           op0=ALU.mult, op1=ALU.add)
            _rsqrt_row(nc, spool, ve, out_rstd, iters=iters)
            return ve

        def stats1(g):
            st = S[g]
            ps_st = psB.tile([P, E], f32, tag="b")
            for ci in range(EC):
                nc.tensor.matmul(
                    ps_st[0:1, 0:TOKG], lhsT=ones_col, rhs=st["x1b"][:, ci, :],
                    start=(ci == 0), stop=(ci == EC - 1),
                )
            # stage raw sums in SBUF via ACT (keeps DVE off the x1c chain);
            # bf16 suffices: mu is small and the LN1 rstd error cancels
            # through LN2's renormalization
            s0r = spool.tile([1, TOKG], bf16, tag="s0r")
            nc.scalar.copy(s0r, ps_st[0:1, 0:TOKG])
            st["s0r"] = s0r
            for ci in range(EC):
                nc.tensor.matmul(
                    ps_st[0:1, TOKG:2 * TOKG], lhsT=ones_col,
                    rhs=st["x1sq"][:, ci, :],
                    start=(ci == 0), stop=(ci == EC - 1),
                )
            s1r = spool.tile([1, TOKG], f32, tag="s1r")
            nc.scalar.copy(s1r, ps_st[0:1, TOKG:2 * TOKG])
            # rstd A: 1 Newton iter (its error cancels through LN2)
            A = spool.tile([1, TOKG], f32, tag="A")
            ve = rowmath(g, s0r, s1r, A, iters=1)
            st["A"] = A
            if has_t1 or has_bsum:
                sd = spool.tile([1, TOKG], bf16, tag="sd")
                nc.vector.tensor_tensor(sd, ve, A, op=ALU.mult)
                st["sd"] = sd

        def mu_b_x1c(g):
            st = S[g]
            ps_mu = psA.tile([P, 2 * TOKG], f32, tag="a")
            nc.tensor.matmul(ps_mu[:, 0:TOKG], lhsT=invE_b, rhs=st["s0r"],
                             start=True, stop=True)
            x1c = x1_pool.tile([P, EC, TOKG], bf16, tag="x1c")
            for ci in range(EC):
                nc.vector.tensor_tensor(
                    x1c[:, ci, :], st["x1b"][:, ci, :], ps_mu[:, 0:TOKG],
                    op=ALU.subtract,
                )
            st["x1c"] = x1c

        def ffn_w1(g):
            st = S[g]
            x1c = st["x1c"]
            hp = h_pool.tile([P, FC, TOKG], bf16, tag="hp")
            for f in range(FC):
                ps = psA.tile([P, 2 * TOKG], f32, tag="a")
                for ci in range(EC):
                    nc.tensor.matmul(
                        ps[:, 0:TOKG],
                        lhsT=w1_sb[:, ci, f * P:(f + 1) * P],
                        rhs=x1c[:, ci, :],
                        start=(ci == 0),
                        stop=(ci == EC - 1) and not has_t1,
                    )
                if has_t1:
                    nc.tensor.matmul(
                        ps[:, 0:TOKG],
                        lhsT=t1_sb[0:1, f * P:(f + 1) * P],
                        rhs=st["sd"],
                        start=False, stop=True,
                    )
                nc.scalar.activation(hp[:, f, :], ps[:, 0:TOKG], ACTF.Relu,
                                     bias=0.0, scale=1.0)
            st["hp"] = hp

        def ffn_w2(g, interleave_stats=False):
            """W2 + diag(g1) residual. For the last group (no later work to
            hide the LN2 chain behind) the stat reductions interleave into
            the chunk stream, using two separate PSUM banks — two
            accumulation groups must never share a bank interleaved."""
            st = S[g]
            x1c, hp = st["x1c"], st["hp"]
            # broadcast A over partitions; stage in SBUF for the DVE mults
            ps_A = psA.tile([P, 2 * TOKG], f32, tag="a")
            nc.tensor.matmul(ps_A[:, 0:TOKG], lhsT=ones_f, rhs=st["A"],
                             start=True, stop=True)
            A_bs = spool.tile([P, TOKG], f32, tag="A_bs")
            pcopy(A_bs, ps_A[:, 0:TOKG])

            x2b = x2_pool.tile([P, EC, TOKG], bf16, tag="x2b")
            x2sq = x2_pool.tile([P, EC, TOKG], bf16, tag="x2sq")
            if interleave_stats:
                ps_sum = psB.tile([P, E], f32, tag="b")
                ps_sq = psB.tile([P, E], f32, tag="b")

            for eo in range(EC):
                ps = psA.tile([P, 2 * TOKG], f32, tag="a")
                for f in range(FC):
                    nc.tensor.matmul(
                        ps[:, 0:TOKG],
                        lhsT=w2_sb[:, f, eo * P:(eo + 1) * P],
                        rhs=hp[:, f, :],
                        start=(f == 0), stop=False,
                    )
                nc.tensor.matmul(
                    ps[:, 0:TOKG],
                    lhsT=dg1_sb[:, eo * P:(eo + 1) * P],
                    rhs=x1c[:, eo, :],
                    start=False, stop=not has_bsum,
                )
                if has_bsum:
                    nc.tensor.matmul(
                        ps[:, 0:TOKG],
                        lhsT=bsr_sb[0:1, eo * P:(eo + 1) * P],
                        rhs=st["sd"],
                        start=False, stop=True,
                    )
                nc.vector.tensor_tensor(x2b[:, eo, :], ps[:, 0:TOKG], A_bs,
                                        op=ALU.mult)
                if interleave_stats:
                    # last group: the sq -> stats -> rowmath chain is the
                    # exposed tail; ACT's square is ~2x faster than Pool's
                    nc.scalar.activation(x2sq[:, eo, :], x2b[:, eo, :],
                                         ACTF.Square, bias=0.0, scale=1.0)
                else:
                    nc.gpsimd.tensor_tensor(x2sq[:, eo, :], x2b[:, eo, :],
                                            x2b[:, eo, :], op=ALU.mult)
                if interleave_stats:
                    if eo >= 1:
                        nc.tensor.matmul(
                            ps_sum[0:1, 0:TOKG], lhsT=ones_col,
                            rhs=x2b[:, eo - 1, :],
                            start=(eo == 1), stop=(eo == EC - 1) and False,
                        )
                    if eo >= 2:
                        nc.tensor.matmul(
                            ps_sq[0:1, 0:TOKG], lhsT=ones_col,
                            rhs=x2sq[:, eo - 2, :],
                            start=(eo == 2), stop=False,
                        )
            if interleave_stats:
                nc.tensor.matmul(ps_sum[0:1, 0:TOKG], lhsT=ones_col,
                                 rhs=x2b[:, EC - 1, :], start=False, stop=True)
                nc.tensor.matmul(ps_sq[0:1, 0:TOKG], lhsT=ones_col,
                                 rhs=x2sq[:, EC - 2, :], start=False,
                                 stop=False)
                nc.tensor.matmul(ps_sq[0:1, 0:TOKG], lhsT=ones_col,
                                 rhs=x2sq[:, EC - 1, :], start=False,
                                 stop=True)
                s0r = spool.tile([1, TOKG], bf16, tag="s0r2")
                nc.scalar.copy(s0r, ps_sum[0:1, 0:TOKG])
                s1r = spool.tile([1, TOKG], f32, tag="s1r2")
                nc.scalar.copy(s1r, ps_sq[0:1, 0:TOKG])
                st["s0r2"], st["s1r2"] = s0r, s1r
            st["x2b"], st["x2sq"] = x2b, x2sq

        def stats2(g):
            st = S[g]
            if "s0r2" not in st:
                ps_st = psB.tile([P, E], f32, tag="b")
                for ci in range(EC):
                    nc.tensor.matmul(
                        ps_st[0:1, 0:TOKG], lhsT=ones_col,
                        rhs=st["x2b"][:, ci, :],
                        start=(ci == 0), stop=(ci == EC - 1),
                    )
                for ci in range(EC):
                    nc.tensor.matmul(
                        ps_st[0:1, TOKG:2 * TOKG], lhsT=ones_col,
                        rhs=st["x2sq"][:, ci, :],
                        start=(ci == 0), stop=(ci == EC - 1),
                    )
                s0r = spool.tile([1, TOKG], bf16, tag="s0r2")
                nc.scalar.copy(s0r, ps_st[0:1, 0:TOKG])
                s1r = spool.tile([1, TOKG], f32, tag="s1r2")
                nc.scalar.copy(s1r, ps_st[0:1, TOKG:2 * TOKG])
                st["s0r2"], st["s1r2"] = s0r, s1r
            # A2 row f32 (applied directly to the output: full precision);
            # C2 shift row bf16 (|C2| ~ |mu2*A2| is small — rounding is
            # negligible, and the bf16 rank-1 broadcast is 4x cheaper)
            A2 = spool1.tile([1, TOKG], f32, tag="bc2")
            # last group: its rowmath chain is fully exposed (no later PE
            # work); 1 Newton iter costs 0.17% rstd error on 1/8 of tokens
            # (~6e-4 globally) against a 4.6x error margin
            rowmath(g, st["s0r2"], st["s1r2"], A2,
                    iters=1 if g == NG - 1 else 2)
            c2 = spool.tile([1, TOKG], bf16, tag="c2")
            nc.vector.scalar_tensor_tensor(c2, st["s0r2"], -INV_E, A2,
                                           op0=ALU.mult, op1=ALU.mult)
            st["A2"], st["c2"] = A2, c2

        def tail_final(g, split=False):
            st = S[g]
            ps_bc = psB.tile([P, E], f32, tag="b")
            nc.tensor.matmul(ps_bc[:, 0:TOKG], lhsT=ones_f, rhs=st["A2"],
                             start=True, stop=True)
            nc.tensor.matmul(ps_bc[:, TOKG:2 * TOKG], lhsT=ones_b,
                             rhs=st["c2"], start=True, stop=True)
            if split:
                # last group: no later PE work hides this chain, so stage
                # the broadcasts in SBUF and fan the chunks out over
                # DVE and Pool in parallel
                bc_s = spool1.tile([P, 2 * TOKG], f32, tag="bc_s")
                nc.scalar.copy(bc_s, ps_bc[:, 0:2 * TOKG])
            ot = out_pool.tile([P, EC, TOKG], bf16, tag="outk")
            for c in range(EC):
                if split and c >= 3:
                    eng, a2s, c2s = (nc.gpsimd, bc_s[:, 0:TOKG],
                                     bc_s[:, TOKG:2 * TOKG])
                else:
                    eng, a2s, c2s = (nc.vector, ps_bc[:, 0:TOKG],
                                     ps_bc[:, TOKG:2 * TOKG])
                eng.tensor_tensor(
                    ot[:, c, :], st["x2b"][:, c, :], a2s, op=ALU.mult,
                )
                eng.tensor_tensor(
                    ot[:, c, :], ot[:, c, :], c2s, op=ALU.add,
                )
                if has_g2b:
                    eng.tensor_scalar(
                        ot[:, c, :], ot[:, c, :], g2_sc[:, c:c + 1],
                        be2_sc[:, c:c + 1], op0=ALU.mult, op1=ALU.add,
                    )
                oq = nc.scalar if (split and c % 2) else nc.sync
                oq.dma_start(
                    io["out"][
                        :,
                        (g * EC + c) * TOKG:(g * EC + c + 1) * TOKG
                    ],
                    ot[:, c, :],
                )

        if has_bsum:
            bsr_sb = singles.tile([1, E], bf16)

        # ---- software-pipelined schedule ----
        # Weave group g's FFN with group g+1's attention and group g+2's
        # projections so PE never drains on the LayerNorm latency chains.
        s1_dma(0)
        s1_dma(1)
        # Issue-delay gadget: the DMA engine serializes all queues, and
        # Pool would otherwise enqueue wk/wv at t=0, beating group-0's
        # inputs in the round-robin. A tiny Pool op gated on the first
        # madd DMA holds their issue until the critical inputs are in
        # flight; wk still lands before the k-projection needs it.
        dly = spool.tile([1, GROUP, P], bf16, tag="dly")
        nc.gpsimd.tensor_copy(dly, S[0]["maddt"])
        nc.gpsimd.dma_start(wk_sb, io["wk"].rearrange("(c p) o -> p c o", p=P))
        nc.gpsimd.dma_start(wv_sb, io["wv"].rearrange("(c p) o -> p c o", p=P))
        nc.gpsimd.dma_start(wo_sb, io["wo"].rearrange("(c p) o -> p c o", p=P))
        nc.gpsimd.dma_start(dg1_sb, io["dg1"])
        nc.gpsimd.dma_start(vecs_sb, io["vecs"])
        if has_t1:
            nc.gpsimd.dma_start(t1_sb, io["t1"])
        if has_bsum:
            nc.gpsimd.dma_start(bsr_sb, io["bsr"])
        nc.sync.dma_start(w1_sb, io["w1g"].rearrange("(c p) o -> p c o", p=P))
        qkproj(0, "q")
        qkproj(0, "k")
        nc.sync.dma_start(w2_sb, io["w2"].rearrange("(c p) o -> p c o", p=P))
        vproj(0)
        attn_energy(0, 0)
        qkproj(1, "q")
        attn_av(0, 0)
        attn_energy(0, 1)
        qkproj(1, "k")
        attn_av(0, 1)
        wo_x1(0)
        stats1(0)
        mu_b_x1c(0)
        vproj(1)
        for g in range(NG):
            if g + 2 < NG:
                s1_dma(g + 2)
            ffn_w1(g)
            ffn_w2(g, interleave_stats=(g == NG - 1))
            if g + 1 < NG:
                attn_energy(g + 1, 0)
                if g + 2 < NG:
                    qkproj(g + 2, "q")
                attn_av(g + 1, 0)
                attn_energy(g + 1, 1)
                if g + 2 < NG:
                    qkproj(g + 2, "k")
                stats2(g)
                if g + 2 < NG:
                    vproj(g + 2, pairs=(0,))
                attn_av(g + 1, 1)
                wo_x1(g + 1)
                stats1(g + 1)
                mu_b_x1c(g + 1)
                if g + 2 < NG:
                    vproj(g + 2, pairs=(1,))
                tail_final(g)
            else:
                stats2(g)
                tail_final(g, split=True)
            S[g] = None


@functools.lru_cache(maxsize=8)
def _build(has_t1=False, has_bsum=False, has_g2b=False, needs_max=False,
           reps=1):
    """reps>1 unrolls the whole kernel body `reps` times in one NEFF.

    Used only for timing: one launch then executes the full computation
    `reps` times back-to-back on-device, so the per-execution time can be
    measured as a slope between two reps values, amortizing the (large,
    kernel-independent) per-launch dispatch overhead of the axon tunnel.
    """
    nc = bacc.Bacc(
        "TRN2", target_bir_lowering=False, debug=False, num_devices=NCORES
    )
    ntok = NPAIRS * P
    io = {
        "xqt": nc.dram_tensor("xqt", [P, EC * ntok], bf16, kind="ExternalInput").ap(),
        "xkt": nc.dram_tensor("xkt", [P, EC * ntok], bf16, kind="ExternalInput").ap(),
        "xvt": nc.dram_tensor("xvt", [P, EC * ntok], bf16, kind="ExternalInput").ap(),
        "madd": nc.dram_tensor("madd", [NPAIRS, P], bf16, kind="ExternalInput").ap(),
        "wq": nc.dram_tensor("wq", [E, E], bf16, kind="ExternalInput").ap(),
        "wk": nc.dram_tensor("wk", [E, E], bf16, kind="ExternalInput").ap(),
        "wv": nc.dram_tensor("wv", [E, E], bf16, kind="ExternalInput").ap(),
        "wo": nc.dram_tensor("wo", [E, E], bf16, kind="ExternalInput").ap(),
        "w1g": nc.dram_tensor("w1g", [E, F], bf16, kind="ExternalInput").ap(),
        "w2": nc.dram_tensor("w2", [F, E], bf16, kind="ExternalInput").ap(),
        "dg1": nc.dram_tensor("dg1", [P, EC * P], bf16, kind="ExternalInput").ap(),
        "vecs": nc.dram_tensor("vecs", [P, 4 * EC], f32, kind="ExternalInput").ap(),
        "out": nc.dram_tensor("out", [P, EC * ntok], bf16, kind="ExternalOutput").ap(),
    }
    if has_t1:
        io["t1"] = nc.dram_tensor("t1", [1, F], f32, kind="ExternalInput").ap()
    if has_bsum:
        io["bsr"] = nc.dram_tensor("bsr", [1, E], f32, kind="ExternalInput").ap()
    with tile.TileContext(nc) as tc:
        for _ in range(reps):
            _emit(tc, io, has_t1, has_bsum, has_g2b, needs_max)
    nc.compile()
    return nc


def _prep(value, key, query, mask, Wv, Wk, Wq, Wo, bo, W1, b1, W2, b2,
          g1, be1, g2, be2):
    """Returns (flags, shared_map, per_core_maps)."""
    bfl = ml_dtypes.bfloat16
    f32n = np.float32
    g1 = np.asarray(g1, f32n)
    t1 = np.asarray(be1, f32n) @ np.asarray(W1, f32n) + np.asarray(b1, f32n)
    bsum = np.asarray(be1, f32n) + np.asarray(b2, f32n)
    has_t1 = bool(np.any(t1 != 0))
    has_bsum = bool(np.any(bsum != 0))
    has_g2b = bool(np.any(np.asarray(g2, f32n) != 1.0)
                   or np.any(np.asarray(be2, f32n) != 0.0))
    # stabilized softmax only needed if some query row is fully masked
    # (otherwise logits are O(5) and raw exp is safe)
    needs_max = bool(np.any(np.all(np.asarray(mask)[:, :, :, 0] == 0, axis=2)))

    w1g = (np.asarray(W1, f32n) * g1[:, None]).astype(bfl)
    dg1 = np.zeros((P, EC * P), f32n)
    idx = np.arange(P)
    for c in range(EC):
        dg1[idx, c * P + idx] = g1[c * P + idx]

    def cols(v):
        return np.asarray(v, f32n).reshape(EC, P).T  # [P, EC]

    vecs = np.concatenate(
        [cols(bo), cols(bsum), cols(g2), cols(be2)], axis=1
    ).astype(f32n)

    shared = {
        "wq": np.ascontiguousarray(np.asarray(Wq, f32n).astype(bfl)),
        "wk": np.ascontiguousarray(np.asarray(Wk, f32n).astype(bfl)),
        "wv": np.ascontiguousarray(np.asarray(Wv, f32n).astype(bfl)),
        "wo": np.ascontiguousarray(np.asarray(Wo, f32n).astype(bfl)),
        "w1g": np.ascontiguousarray(w1g),
        "w2": np.ascontiguousarray(np.asarray(W2, f32n).astype(bfl)),
        "dg1": np.ascontiguousarray(dg1.astype(bfl)),
        "vecs": np.ascontiguousarray(vecs),
    }
    if has_t1:
        shared["t1"] = np.ascontiguousarray(t1.reshape(1, F).astype(bfl))
    if has_bsum:
        shared["bsr"] = np.ascontiguousarray(bsum.reshape(1, E).astype(bfl))

    def emaj(x2d):
        # [ntok, E] f32 -> [P, NG, EC, TOKG] bf16 -> flat [P, EC*ntok]
        a = np.asarray(x2d, f32n).reshape(NG, TOKG, EC, P)
        return np.ascontiguousarray(
            a.transpose(3, 0, 2, 1).astype(bfl).reshape(P, EC * NG * TOKG)
        )

    npc = 64 // NCORES
    ntok = NPAIRS * P
    in_maps = []
    for c in range(NCORES):
        nsl = slice(c * npc, (c + 1) * npc)
        madd = np.where(
            np.asarray(mask)[nsl, :, :, 0] == 0, f32n(-1e20), f32n(0.0)
        ).reshape(NPAIRS, P).astype(bfl)
        in_maps.append(
            {
                "xqt": emaj(np.asarray(query)[nsl].reshape(ntok, E)),
                "xkt": emaj(np.asarray(key)[nsl].reshape(ntok, E)),
                "xvt": emaj(np.asarray(value)[nsl].reshape(ntok, E)),
                "madd": np.ascontiguousarray(madd),
                **shared,
            }
        )
    return (has_t1, has_bsum, has_g2b, needs_max), in_maps


def _prep_in_maps(**inputs):
    flags, in_maps = _prep(**{k: np.asarray(v) for k, v in inputs.items()})
    return in_maps


def _prep_flags(**inputs):
    flags, in_maps = _prep(**{k: np.asarray(v) for k, v in inputs.items()})
    return flags


def _gather(res):
    outs = []
    for r in res.results:
        a = r["out"].reshape(P, NG, EC, TOKG)
        outs.append(a.transpose(1, 3, 2, 0).reshape(NPAIRS * P, E))
    out = np.concatenate(outs, axis=0)
    return out.reshape(64, 2, P, E).astype(np.float32)


def gather_concat(out_concat):
    """Reassemble a [NCORES*P, EC*ntok] concatenated raw output (as produced
    by per-core sharding along axis 0) into the full [64, 2, P, E] result."""
    a = np.asarray(out_concat).reshape(NCORES, P, NG, EC, TOKG)
    a = a.transpose(0, 2, 4, 3, 1).reshape(NCORES * NPAIRS * P, E)
    return a.reshape(64, 2, P, E).astype(np.float32)


def kernel(**inputs) -> np.ndarray:
    flags, in_maps = _prep(**{k: np.asarray(v) for k, v in inputs.items()})
    nc = _build(*flags)
    res = run_bass_kernel_spmd(nc, in_maps, core_ids=list(range(NCORES)))
    return _gather(res)


def run_traced(**inputs):
    flags, in_maps = _prep(**{k: np.asarray(v) for k, v in inputs.items()})
    nc = _build(*flags)
    res = run_bass_kernel_spmd(
        nc, in_maps, core_ids=list(range(NCORES)), trace=True
    )
    return _gather(res), res



# revision 24
# speedup vs baseline: 213.3498x; 1.0852x over previous
"""Trainium2 Bass kernel for nn_EntailmentTransformerBlock — v3 (512-token
supergroups).

Single fused pass per 512-token supergroup (4 (n,s) pairs), all activations
E-major ([E_chunk=128 partitions, tokens]) end-to-end:
  - Inputs pre-transposed to E-major bf16 on the host (no PE transposes).
  - All token-streaming matmuls (QKV/O projections, FFN, LayerNorm stats,
    row broadcasts) run 512 columns per instruction — half the instruction
    count of a 256-token grouping for the same column-cycles, which matters
    because real HW pays ~20 ns of issue overhead per instruction that the
    cost model does not show.  Only the per-pair attention internals
    (energy / AV, inherently 128-token blocks) stay at 128 columns.
  - LayerNorm statistics via PE partition-reduction matmuls (ones column);
    rstd via DVE fast-inverse-sqrt (bit trick + Newton steps) so ACT never
    loads the Sqrt table (Exp/Relu/Square share one table).
  - LN1 is folded into the FFN: W1 is pre-scaled by gamma1 on the host,
    the per-token rstd A commutes through ReLU (A>0) and is applied once
    after W2; the residual-stream gamma1*(x1-mu) enters the W2 PSUM via
    5 diagonal-block matmuls.
  - Per-token rows (mu, rstd, ...) are broadcast across partitions with
    rank-1 f32 matmuls (ones ⊗ row).
  - Software-pipelined emission: FFN of supergroup s interleaves with
    attention of s+1 and input projections of s+2, keeping PE busy across
    the LayerNorm dependency chains.
Sharding: pure data-parallel over batch n (64) across 8 cores.
"""

import functools

import numpy as np
import ml_dtypes

import concourse.bass as bass
import concourse.tile as tile
from concourse import bacc, mybir
from concourse.bass_utils import run_bass_kernel_spmd

P = 128
E = 640
EC = 5           # E / 128 chunks
F = 2560
FC = 20          # F / 128 chunks
H = 5            # heads, head_dim = 128
NCORES = 8
NPAIRS = 16      # (n, s) pairs per core: 8 n * 2 s
GROUP = 4        # pairs per processing supergroup
NG = NPAIRS // GROUP
TOKG = GROUP * P  # tokens per supergroup = 512
EPS = 1e-5
SCALE = float(1.0 / np.sqrt(128.0))  # 1/sqrt(key_len)
INV_E = 1.0 / float(E)
MAGIC = 0x5F3759DF

f32 = mybir.dt.float32
bf16 = mybir.dt.bfloat16
i32 = mybir.dt.int32

AX = mybir.AxisListType.X
ALU = mybir.AluOpType
ACTF = mybir.ActivationFunctionType


def _rsqrt_row(nc, spool, ve, out, iters=2):
    """out = 1/sqrt(ve) elementwise on a [1, N] f32 row (DVE only).

    Fast-inverse-sqrt bit trick + Newton iterations (1 iter ~1.7e-3,
    2 iters ~5e-6 rel err).
    """
    n = ve.shape[-1]
    # bit trick staged in-place through `out` (no scratch row):
    # out_i = ve_i >> 1 ; out_i = (out_i - MAGIC) * -1 = MAGIC - out_i
    nc.vector.tensor_scalar(out.bitcast(i32), ve.bitcast(i32), 1, None,
                            op0=ALU.logical_shift_right)
    nc.vector.tensor_scalar(out.bitcast(i32), out.bitcast(i32), MAGIC, -1,
                            op0=ALU.subtract, op1=ALU.mult)
    for it in range(iters):
        # y <- y*(1.5 - 0.5*v*y^2), fused into 3 DVE ops
        t1 = spool.tile([1, n], f32, tag="rsq_n")
        nc.vector.tensor_tensor(t1, ve, out, op=ALU.mult)
        nc.vector.scalar_tensor_tensor(t1, t1, -0.5, out,
                                       op0=ALU.mult, op1=ALU.mult)
        nc.vector.scalar_tensor_tensor(out, t1, 1.5, out,
                                       op0=ALU.add, op1=ALU.mult)


def _emit(tc, io, has_t1, has_bsum, has_g2b, needs_max, has_g1):
    nc = tc.nc
    from contextlib import ExitStack

    with ExitStack() as ctx:
        singles = ctx.enter_context(tc.tile_pool(name="singles", bufs=1))
        qin_pool = ctx.enter_context(tc.tile_pool(name="qin", bufs=2))
        kin_pool = ctx.enter_context(tc.tile_pool(name="kin", bufs=2))
        vin_pool = ctx.enter_context(tc.tile_pool(name="vin", bufs=2))
        madd_pool = ctx.enter_context(tc.tile_pool(name="madd", bufs=2))
        qkt_pool = ctx.enter_context(tc.tile_pool(name="qkt", bufs=2))
        vtok_pool = ctx.enter_context(tc.tile_pool(name="vtok", bufs=2))
        outt_pool = ctx.enter_context(tc.tile_pool(name="outt", bufs=1))
        attn_pool = ctx.enter_context(tc.tile_pool(name="attn", bufs=2))
        x1b_pool = ctx.enter_context(tc.tile_pool(name="x1b", bufs=1))
        x1c_pool = ctx.enter_context(tc.tile_pool(name="x1c", bufs=1))
        sq_pool = ctx.enter_context(tc.tile_pool(name="sq", bufs=1))
        h_pool = ctx.enter_context(tc.tile_pool(name="h", bufs=1))
        x2_pool = ctx.enter_context(tc.tile_pool(name="x2", bufs=1))
        # per-pair softmax statistics ([P, H] — tiny)
        hpool = ctx.enter_context(tc.tile_pool(name="hstats", bufs=4))
        # per-supergroup [1, TOKG] rows and broadcast staging
        spool = ctx.enter_context(tc.tile_pool(name="rows", bufs=1))
        # large row tiles with within-iteration lifetime only
        spool1 = ctx.enter_context(tc.tile_pool(name="rows1", bufs=1))
        # PSUM: psA = 4 x 1-bank [P, 512] f32 tiles for token-streaming
        # chains; psB = 2 x 2-bank [P, 1024] f32 tiles for attention /
        # vproj / stats -> 8 banks total.
        psA = ctx.enter_context(tc.tile_pool(name="psA", bufs=4, space="PSUM"))
        psB = ctx.enter_context(tc.tile_pool(name="psB", bufs=2, space="PSUM"))

        # ---- weights / constants (resident) ----
        # DMA issue order matters: the HBM pipe is effectively serial, so
        # group-0 inputs must land right after wq/wk; w1g/w2 are deferred
        # (emitted after the prologue) since the FFN starts ~20us in.
        wq_sb = singles.tile([P, EC, E], bf16)
        nc.sync.dma_start(wq_sb, io["wq"].rearrange("(c p) o -> p c o", p=P))
        wk_sb = singles.tile([P, EC, E], bf16)
        wv_sb = singles.tile([P, EC, E], bf16)
        wo_sb = singles.tile([P, EC, E], bf16)
        w1_sb = singles.tile([P, EC, F], bf16)
        w2_sb = singles.tile([P, FC, E], bf16)
        if has_g1:
            dg1_sb = singles.tile([P, EC * P], bf16)
        vecs_sb = singles.tile([P, 4 * EC], f32)
        bo_sc = vecs_sb[:, 0:EC]
        bsum_sc = vecs_sb[:, EC:2 * EC]
        g2_sc = vecs_sb[:, 2 * EC:3 * EC]
        be2_sc = vecs_sb[:, 3 * EC:4 * EC]
        if has_t1:
            t1_sb = singles.tile([1, F], bf16)

        ones_f = singles.tile([1, P], f32)
        nc.vector.memset(ones_f, 1.0)
        ones_b = singles.tile([1, P], bf16)
        nc.vector.memset(ones_b, 1.0)
        # 1/E row: folds the mean division into the rank-1 broadcast matmul
        invE_b = singles.tile([1, P], bf16)
        nc.vector.memset(invE_b, INV_E)
        # ones column for M=1 partition reductions
        ones_col = singles.tile([P, 1], bf16)
        nc.vector.memset(ones_col, 1.0)

        ncopy = 0

        def pcopy(dst, src):
            nonlocal ncopy
            ncopy += 1
            if ncopy % 2:
                nc.vector.tensor_copy(dst, src)
            else:
                nc.scalar.copy(dst, src)

        def in_slice(name, g):
            return io[name][:, g * EC * TOKG:(g + 1) * EC * TOKG].rearrange(
                "p (c t) -> p c t", c=EC
            )

        # per-supergroup state passed between pipeline stages
        S = [dict() for _ in range(NG)]

        def s1_dma(g):
            """Issue the supergroup's input DMAs (hardware DGE queues)."""
            st = S[g]
            tq = qin_pool.tile([P, EC, TOKG], bf16, tag="qin")
            tk = kin_pool.tile([P, EC, TOKG], bf16, tag="kin")
            if g == 0:
                # group 0 rides the scalar queue so it isn't serialized
                # behind wq on sync
                nc.scalar.dma_start(tq, in_slice("xqt", g))
            else:
                nc.sync.dma_start(tq, in_slice("xqt", g))
            nc.scalar.dma_start(tk, in_slice("xkt", g))
            tv = vin_pool.tile([P, EC, TOKG], bf16, tag="vin")
            nc.scalar.dma_start(tv, in_slice("xvt", g))
            maddt = madd_pool.tile([1, GROUP, P], bf16, tag="madd")
            nc.sync.dma_start(
                maddt,
                io["madd"][g * GROUP:(g + 1) * GROUP, :].rearrange(
                    "(o a) b -> o a b", o=1
                ),
            )
            st["xqT"], st["xkT"], st["xvT"], st["maddt"] = tq, tk, tv, maddt

        def qkproj(g, which):
            st = S[g]
            srcT = st["xqT"] if which == "q" else st["xkT"]
            w_sb = wq_sb if which == "q" else wk_sb
            dst = qkt_pool.tile([P, EC, TOKG], bf16, tag=f"{which}Tb")
            for eo in range(EC):
                ps = psA.tile([P, TOKG], f32, tag="a")
                for ci in range(EC):
                    nc.tensor.matmul(
                        ps,
                        lhsT=w_sb[:, ci, eo * P:(eo + 1) * P],
                        rhs=srcT[:, ci, :],
                        start=(ci == 0),
                        stop=(ci == EC - 1),
                    )
                pcopy(dst[:, eo, :], ps)
            st[f"{which}Tb"] = dst

        def vproj(g, pairs=None):
            st = S[g]
            tv = st["xvT"]
            if pairs is None:
                pairs = tuple(range(GROUP))
            if 0 in pairs:
                v_tok = vtok_pool.tile([P, GROUP, E], bf16, tag="v_tok")
                st["v_tok"] = v_tok
            v_tok = st["v_tok"]
            for pr in pairs:
                ps = psB.tile([P, 2 * TOKG], f32, tag="b")
                for n0, nsz in ((0, 512), (512, 128)):
                    for ci in range(EC):
                        nc.tensor.matmul(
                            ps[:, n0:n0 + nsz],
                            lhsT=tv[:, ci, pr * P:(pr + 1) * P],
                            rhs=wv_sb[:, ci, n0:n0 + nsz],
                            start=(ci == 0),
                            stop=(ci == EC - 1),
                        )
                pcopy(v_tok[:, pr, :], ps[:, 0:E])

        def attn_energy(g, pr):
            """Energy -> masked softmax(axis=q) numerator + denominator."""
            st = S[g]
            if pr == 0:
                outT = outt_pool.tile([P, H, TOKG], bf16, tag="outT")
                st["outT"] = outT
            qTb, kTb, maddt = st["qTb"], st["kTb"], st["maddt"]
            tsl = slice(pr * P, (pr + 1) * P)
            psb = psB.tile([P, 2 * TOKG], f32, tag="b")
            pse5 = psb[:, 0:H * P].rearrange("p (h q) -> p h q", h=H)
            for h in range(H):
                nc.tensor.matmul(
                    pse5[:, h, :], lhsT=kTb[:, h, tsl], rhs=qTb[:, h, tsl],
                    start=True, stop=False,
                )
                nc.tensor.matmul(
                    pse5[:, h, :], lhsT=ones_b, rhs=maddt[:, pr, :],
                    start=False, stop=True,
                )
            attn5 = attn_pool.tile([P, H, P], bf16, tag="asb")
            ssum5 = hpool.tile([P, H], f32, tag="ssum5")
            if needs_max:
                # stabilized softmax (only needed if a row can be fully
                # masked: logits are otherwise O(5), well within exp range)
                mx5 = hpool.tile([P, H], f32, tag="mx5")
                nc.vector.reduce_max(out=mx5, in_=pse5, axis=AX)
                negb5 = hpool.tile([P, H], f32, tag="negb5")
                nc.vector.tensor_scalar_mul(negb5, mx5, -SCALE)
                for h in range(H):
                    nc.scalar.activation(
                        attn5[:, h, :], pse5[:, h, :], ACTF.Exp,
                        bias=negb5[:, h:h + 1], scale=SCALE,
                        accum_out=ssum5[:, h:h + 1],
                    )
            else:
                nc.scalar.activation(attn5, pse5, ACTF.Exp,
                                     bias=0.0, scale=SCALE)
                nc.vector.reduce_sum(out=ssum5, in_=attn5, axis=AX)
            rec5 = hpool.tile([P, H], f32, tag="rec5")
            nc.vector.reciprocal(rec5, ssum5)
            st[f"attn{pr}"], st[f"rec{pr}"] = attn5, rec5

        def attn_av(g, pr):
            """Normalize and apply attention to v for one pair."""
            st = S[g]
            tsl = slice(pr * P, (pr + 1) * P)
            attn5, rec5 = st[f"attn{pr}"], st[f"rec{pr}"]
            # normalize on Pool (SBUF-only) — keeps DVE clear for the
            # x1b/x1c epilogues that gate the next FFN
            nc.gpsimd.tensor_tensor(
                attn5, attn5, rec5[:, :, None].to_broadcast([P, H, P]),
                op=ALU.mult,
            )
            psb = psB.tile([P, 2 * TOKG], f32, tag="b")
            pso5 = psb[:, 0:H * P].rearrange("p (h q) -> p h q", h=H)
            for h in range(H):
                nc.tensor.matmul(
                    pso5[:, h, :],
                    lhsT=S[g]["v_tok"][:, pr, h * P:(h + 1) * P],
                    rhs=attn5[:, h, :],
                    start=True, stop=True,
                )
            # alternate the copy engine so neither ACT nor DVE owns all of
            # the attention drain work
            if pr % 2:
                nc.scalar.copy(st["outT"][:, :, tsl], pso5)
            else:
                nc.vector.tensor_copy(st["outT"][:, :, tsl], pso5)

        def wo_x1(g):
            """Wo projection + bo + residual -> x1 (bf16) + Pool squares."""
            st = S[g]
            outT = st["outT"]
            x1b = x1b_pool.tile([P, EC, TOKG], bf16, tag="x1b")
            x1sq = sq_pool.tile([P, EC, TOKG], bf16, tag="x1sq")
            for eo in range(EC):
                ps = psA.tile([P, TOKG], f32, tag="a")
                for h in range(H):
                    nc.tensor.matmul(
                        ps,
                        lhsT=wo_sb[:, h, eo * P:(eo + 1) * P],
                        rhs=outT[:, h, :],
                        start=(h == 0),
                        stop=(h == H - 1),
                    )
                nc.vector.scalar_tensor_tensor(
                    x1b[:, eo, :], ps, bo_sc[:, eo:eo + 1],
                    st["xqT"][:, eo, :], op0=ALU.add, op1=ALU.add,
                )
                # square on the idle Pool engine (SBUF-only), per chunk
                nc.gpsimd.tensor_tensor(x1sq[:, eo, :], x1b[:, eo, :],
                                        x1b[:, eo, :], op=ALU.mult)
            st["x1b"], st["x1sq"] = x1b, x1sq

        def rowmath(g, s0r, s1r, out_rstd, iters):
            """rstd = 1/sqrt(sum1/E + eps - (sum0/E)^2) from raw-sum rows."""
            # m2 in bf16: mu^2 is ~1e-3 of the variance, rounding is noise
            m2 = spool.tile([1, TOKG], bf16, tag="m2")
            nc.vector.tensor_tensor(m2, s0r, s0r, op=ALU.mult)
            ve = spool.tile([1, TOKG], f32, tag="vep")
            nc.vector.tensor_scalar(ve, s1r, INV_E, EPS,
                                    op0=ALU.mult, op1=ALU.add)
            nc.vector.scalar_tensor_tensor(ve, m2, -INV_E * INV_E, ve,
                                           op0=ALU.mult, op1=ALU.add)
            _rsqrt_row(nc, spool, ve, out_rstd, iters=iters)
            return ve

        def stats1(g):
            st = S[g]
            ps_st = psB.tile([P, 2 * TOKG], f32, tag="b")
            for ci in range(EC):
                nc.tensor.matmul(
                    ps_st[0:1, 0:TOKG], lhsT=ones_col, rhs=st["x1b"][:, ci, :],
                    start=(ci == 0), stop=(ci == EC - 1),
                )
            # stage raw sums in SBUF via ACT (keeps DVE off the x1c chain);
            # bf16 suffices: mu is small and the LN1 rstd error cancels
            # through LN2's renormalization
            s0r = spool.tile([1, TOKG], bf16, tag="s0r")
            nc.scalar.copy(s0r, ps_st[0:1, 0:TOKG])
            st["s0r"] = s0r
            for ci in range(EC):
                nc.tensor.matmul(
                    ps_st[0:1, TOKG:2 * TOKG], lhsT=ones_col,
                    rhs=st["x1sq"][:, ci, :],
                    start=(ci == 0), stop=(ci == EC - 1),
                )
            s1r = spool.tile([1, TOKG], bf16, tag="s1r")
            nc.scalar.copy(s1r, ps_st[0:1, TOKG:2 * TOKG])
            # rstd A: 1 Newton iter (its error cancels through LN2)
            A = spool.tile([1, TOKG], f32, tag="A")
            ve = rowmath(g, s0r, s1r, A, iters=1)
            st["A"] = A
            if has_t1 or has_bsum:
                sd = spool.tile([1, TOKG], bf16, tag="sd")
                nc.vector.tensor_tensor(sd, ve, A, op=ALU.mult)
                st["sd"] = sd

        def mu_b_x1c(g):
            st = S[g]
            ps_mu = psA.tile([P, TOKG], f32, tag="a")
            nc.tensor.matmul(ps_mu, lhsT=invE_b, rhs=st["s0r"],
                             start=True, stop=True)
            x1c = x1c_pool.tile([P, EC, TOKG], bf16, tag="x1c")
            for ci in range(EC):
                nc.vector.tensor_tensor(
                    x1c[:, ci, :], st["x1b"][:, ci, :], ps_mu,
                    op=ALU.subtract,
                )
            st["x1c"] = x1c

        def ffn_w1(g):
            st = S[g]
            x1c = st["x1c"]
            hp = h_pool.tile([P, FC, TOKG], bf16, tag="hp")
            for f in range(FC):
                ps = psA.tile([P, TOKG], f32, tag="a")
                for ci in range(EC):
                    nc.tensor.matmul(
                        ps,
                        lhsT=w1_sb[:, ci, f * P:(f + 1) * P],
                        rhs=x1c[:, ci, :],
                        start=(ci == 0),
                        stop=(ci == EC - 1) and not has_t1,
                    )
                if has_t1:
                    nc.tensor.matmul(
                        ps,
                        lhsT=t1_sb[0:1, f * P:(f + 1) * P],
                        rhs=st["sd"],
                        start=False, stop=True,
                    )
                nc.scalar.activation(hp[:, f, :], ps, ACTF.Relu,
                                     bias=0.0, scale=1.0)
            st["hp"] = hp

        def ffn_w2(g, interleave_stats=False, fillers=None):
            """W2 + diag(g1) residual. For the last supergroup (no later
            work to hide the LN2 chain behind) the stat reductions
            interleave into the chunk stream, using two separate PSUM
            tiles — two accumulation groups must never share a bank
            interleaved."""
            st = S[g]
            x1c, hp = st["x1c"], st["hp"]
            # broadcast A over partitions; stage in SBUF for the DVE mults
            # bf16 rank-1 broadcast (per-token error in A cancels through
            # LN2); the f32->bf16 row conversion is one tiny DVE op
            A_b = spool.tile([1, TOKG], bf16, tag="A_b")
            nc.vector.tensor_copy(A_b, st["A"])
            ps_A = psA.tile([P, TOKG], f32, tag="a")
            nc.tensor.matmul(ps_A, lhsT=ones_b, rhs=A_b,
                             start=True, stop=True)
            A_bs = spool.tile([P, TOKG], bf16, tag="A_bs")
            pcopy(A_bs, ps_A)

            x2b = x2_pool.tile([P, EC, TOKG], bf16, tag="x2b")
            x2sq = sq_pool.tile([P, EC, TOKG], bf16, tag="x2sq")
            if interleave_stats:
                ps_sum = psB.tile([P, 2 * TOKG], f32, tag="b")
                ps_sq = psB.tile([P, 2 * TOKG], f32, tag="b")

            for eo in range(EC):
                ps = psA.tile([P, TOKG], f32, tag="a")
                for f in range(FC):
                    nc.tensor.matmul(
                        ps,
                        lhsT=w2_sb[:, f, eo * P:(eo + 1) * P],
                        rhs=hp[:, f, :],
                        start=(f == 0),
                        stop=(f == FC - 1) and not (has_g1 or has_bsum),
                    )
                if has_g1:
                    nc.tensor.matmul(
                        ps,
                        lhsT=dg1_sb[:, eo * P:(eo + 1) * P],
                        rhs=x1c[:, eo, :],
                        start=False, stop=not has_bsum,
                    )
                if has_bsum:
                    nc.tensor.matmul(
                        ps,
                        lhsT=bsr_sb[0:1, eo * P:(eo + 1) * P],
                        rhs=st["sd"],
                        start=False, stop=True,
                    )
                if has_g1:
                    nc.vector.tensor_tensor(x2b[:, eo, :], ps, A_bs,
                                            op=ALU.mult)
                else:
                    # g1 == 1: residual enters on DVE instead of a
                    # diagonal-block matmul (saves 512 PE columns per chunk)
                    nc.vector.tensor_tensor(x2b[:, eo, :], ps,
                                            x1c[:, eo, :], op=ALU.add)
                    nc.vector.tensor_tensor(x2b[:, eo, :], x2b[:, eo, :],
                                            A_bs, op=ALU.mult)
                if interleave_stats:
                    # last supergroup: the sq -> stats -> rowmath chain is
                    # the exposed tail; ACT's square is ~2x faster than
                    # Pool's
                    nc.scalar.activation(x2sq[:, eo, :], x2b[:, eo, :],
                                         ACTF.Square, bias=0.0, scale=1.0)
                else:
                    nc.gpsimd.tensor_tensor(x2sq[:, eo, :], x2b[:, eo, :],
                                            x2b[:, eo, :], op=ALU.mult)
                if interleave_stats:
                    if eo >= 1:
                        nc.tensor.matmul(
                            ps_sum[0:1, 0:TOKG], lhsT=ones_col,
                            rhs=x2b[:, eo - 1, :],
                            start=(eo == 1), stop=False,
                        )
                    if eo >= 2:
                        nc.tensor.matmul(
                            ps_sq[0:1, 0:TOKG], lhsT=ones_col,
                            rhs=x2sq[:, eo - 2, :],
                            start=(eo == 2), stop=False,
                        )
                if fillers is not None and eo < len(fillers):
                    # attention work for s+1 slots between the ~4.5 us W2
                    # chunk chains — each softmax latency chain is covered
                    # by the next chunk's matmuls
                    fillers[eo]()
            if interleave_stats:
                nc.tensor.matmul(ps_sum[0:1, 0:TOKG], lhsT=ones_col,
                                 rhs=x2b[:, EC - 1, :], start=False, stop=True)
                nc.tensor.matmul(ps_sq[0:1, 0:TOKG], lhsT=ones_col,
                                 rhs=x2sq[:, EC - 2, :], start=False,
                                 stop=False)
                nc.tensor.matmul(ps_sq[0:1, 0:TOKG], lhsT=ones_col,
                                 rhs=x2sq[:, EC - 1, :], start=False,
                                 stop=True)
                s0r = spool.tile([1, TOKG], bf16, tag="s0r2")
                nc.scalar.copy(s0r, ps_sum[0:1, 0:TOKG])
                s1r = spool.tile([1, TOKG], bf16, tag="s1r2")
                nc.scalar.copy(s1r, ps_sq[0:1, 0:TOKG])
                st["s0r2"], st["s1r2"] = s0r, s1r
            st["x2b"], st["x2sq"] = x2b, x2sq

        def stats2(g):
            st = S[g]
            if "s0r2" not in st:
                ps_st = psB.tile([P, 2 * TOKG], f32, tag="b")
                for ci in range(EC):
                    nc.tensor.matmul(
                        ps_st[0:1, 0:TOKG], lhsT=ones_col,
                        rhs=st["x2b"][:, ci, :],
                        start=(ci == 0), stop=(ci == EC - 1),
                    )
                for ci in range(EC):
                    nc.tensor.matmul(
                        ps_st[0:1, TOKG:2 * TOKG], lhsT=ones_col,
                        rhs=st["x2sq"][:, ci, :],
                        start=(ci == 0), stop=(ci == EC - 1),
                    )
                s0r = spool.tile([1, TOKG], bf16, tag="s0r2")
                nc.scalar.copy(s0r, ps_st[0:1, 0:TOKG])
                s1r = spool.tile([1, TOKG], bf16, tag="s1r2")
                nc.scalar.copy(s1r, ps_st[0:1, TOKG:2 * TOKG])
                st["s0r2"], st["s1r2"] = s0r, s1r
            # A2 row f32 (applied directly to the output: full precision);
            # C2 shift row bf16 (|C2| ~ |mu2*A2| is small — rounding is
            # negligible, and the bf16 rank-1 broadcast is 4x cheaper)
            A2 = spool1.tile([1, TOKG], f32, tag="bc2")
            # last supergroup: its rowmath chain is fully exposed (no later
            # PE work); 1 Newton iter costs 0.17% rstd error on 1/4 of
            # tokens (~1e-3 globally) against a 4.6x error margin
            rowmath(g, st["s0r2"], st["s1r2"], A2,
                    iters=1 if g == NG - 1 else 2)
            c2 = spool.tile([1, TOKG], bf16, tag="c2")
            nc.vector.scalar_tensor_tensor(c2, st["s0r2"], -INV_E, A2,
                                           op0=ALU.mult, op1=ALU.mult)
            st["A2"], st["c2"] = A2, c2

        def tail_final(g, split=False):
            st = S[g]
            A2b = spool.tile([1, TOKG], bf16, tag="A_b")
            nc.vector.tensor_copy(A2b, st["A2"])
            ps_bc = psB.tile([P, 2 * TOKG], f32, tag="b")
            nc.tensor.matmul(ps_bc[:, 0:TOKG], lhsT=ones_b, rhs=A2b,
                             start=True, stop=True)
            nc.tensor.matmul(ps_bc[:, TOKG:2 * TOKG], lhsT=ones_b,
                             rhs=st["c2"], start=True, stop=True)
            if split:
                # last supergroup: no later PE work hides this chain, so
                # stage the broadcasts in SBUF and fan the chunks out over
                # DVE and Pool in parallel
                # reuse slots that are dead by the final tail: A_bs (last
                # read in ffn_w2) and x1sq (last read in stats1)
                a2_s = spool.tile([P, TOKG], bf16, tag="A_bs")
                nc.scalar.copy(a2_s, ps_bc[:, 0:TOKG])
                c2_s = sq_pool.tile([P, TOKG], bf16, tag="x1sq")
                nc.scalar.copy(c2_s, ps_bc[:, TOKG:2 * TOKG])
            # scale/shift x2b in place (it is dead after this) and DMA
            # straight from it — no separate output staging tile
            ot = st["x2b"]
            for c in range(EC):
                if split and c >= 3:
                    eng, a2s, c2s = nc.gpsimd, a2_s, c2_s
                else:
                    eng, a2s, c2s = (nc.vector, ps_bc[:, 0:TOKG],
                                     ps_bc[:, TOKG:2 * TOKG])
                eng.tensor_tensor(
                    ot[:, c, :], ot[:, c, :], a2s, op=ALU.mult,
                )
                eng.tensor_tensor(
                    ot[:, c, :], ot[:, c, :], c2s, op=ALU.add,
                )
                if has_g2b:
                    eng.tensor_scalar(
                        ot[:, c, :], ot[:, c, :], g2_sc[:, c:c + 1],
                        be2_sc[:, c:c + 1], op0=ALU.mult, op1=ALU.add,
                    )
                oq = nc.scalar if (split and c % 2) else nc.sync
                oq.dma_start(
                    io["out"][
                        :,
                        (g * EC + c) * TOKG:(g * EC + c + 1) * TOKG
                    ],
                    ot[:, c, :],
                )

        if has_bsum:
            bsr_sb = singles.tile([1, E], bf16)

        # ---- software-pipelined schedule ----
        # Weave supergroup s's FFN with s+1's attention and s+2's
        # projections so PE never drains on the LayerNorm latency chains.
        s1_dma(0)
        s1_dma(1)
        # Issue-delay gadget: the DMA engine serializes all queues, and
        # Pool would otherwise enqueue wk/wv at t=0, beating group-0's
        # inputs in the round-robin. A tiny Pool op gated on the first
        # madd DMA holds their issue until the critical inputs are in
        # flight; wk still lands before the k-projection needs it.
        dly = spool.tile([1, GROUP * P], bf16, tag="c2")
        nc.gpsimd.tensor_copy(dly, S[0]["maddt"].rearrange("o a b -> o (a b)"))
        nc.gpsimd.dma_start(wk_sb, io["wk"].rearrange("(c p) o -> p c o", p=P))
        nc.gpsimd.dma_start(wv_sb, io["wv"].rearrange("(c p) o -> p c o", p=P))
        nc.gpsimd.dma_start(wo_sb, io["wo"].rearrange("(c p) o -> p c o", p=P))
        if has_g1:
            nc.gpsimd.dma_start(dg1_sb, io["dg1"])
        nc.gpsimd.dma_start(vecs_sb, io["vecs"])
        if has_t1:
            nc.gpsimd.dma_start(t1_sb, io["t1"])
        if has_bsum:
            nc.gpsimd.dma_start(bsr_sb, io["bsr"])
        nc.sync.dma_start(w1_sb, io["w1g"].rearrange("(c p) o -> p c o", p=P))
        qkproj(0, "q")
        qkproj(0, "k")
        nc.sync.dma_start(w2_sb, io["w2"].rearrange("(c p) o -> p c o", p=P))
        vproj(0)
        attn_energy(0, 0)
        attn_energy(0, 1)
        qkproj(1, "q")
        attn_av(0, 0)
        attn_energy(0, 2)
        attn_av(0, 1)
        qkproj(1, "k")
        attn_energy(0, 3)
        attn_av(0, 2)
        vproj(1, (0, 1))
        attn_av(0, 3)
        wo_x1(0)
        stats1(0)
        mu_b_x1c(0)
        vproj(1, (2, 3))
        for g in range(NG):
            if g + 2 < NG:
                s1_dma(g + 2)
            ffn_w1(g)
            if g + 1 < NG:
                s1 = g + 1

                def mk(*fns):
                    def run(fns=fns):
                        for f in fns:
                            f()
                    return run

                fillers = [
                    mk(lambda: attn_energy(s1, 0), lambda: attn_energy(s1, 1)),
                    mk(lambda: attn_av(s1, 0), lambda: attn_energy(s1, 2)),
                    mk(lambda: attn_av(s1, 1), lambda: attn_energy(s1, 3)),
                    mk(lambda: attn_av(s1, 2)),
                    mk(lambda: attn_av(s1, 3)),
                ]
                ffn_w2(g, fillers=fillers)
                if g + 2 < NG:
                    qkproj(g + 2, "q")
                    qkproj(g + 2, "k")
                stats2(g)
                wo_x1(g + 1)
                stats1(g + 1)
                mu_b_x1c(g + 1)
                if g + 2 < NG:
                    vproj(g + 2)
                tail_final(g)
            else:
                ffn_w2(g, interleave_stats=True)
                stats2(g)
                tail_final(g, split=True)
            S[g] = None


@functools.lru_cache(maxsize=8)
def _build(has_t1=False, has_bsum=False, has_g2b=False, needs_max=False,
           has_g1=False, reps=1):
    """reps>1 unrolls the whole kernel body `reps` times in one NEFF.

    Used only for timing: one launch then executes the full computation
    `reps` times back-to-back on-device, so the per-execution time can be
    measured as a slope between two reps values, amortizing the (large,
    kernel-independent) per-launch dispatch overhead of the axon tunnel.
    """
    nc = bacc.Bacc(
        "TRN2", target_bir_lowering=False, debug=False, num_devices=NCORES
    )
    ntok = NPAIRS * P
    io = {
        "xqt": nc.dram_tensor("xqt", [P, EC * ntok], bf16, kind="ExternalInput").ap(),
        "xkt": nc.dram_tensor("xkt", [P, EC * ntok], bf16, kind="ExternalInput").ap(),
        "xvt": nc.dram_tensor("xvt", [P, EC * ntok], bf16, kind="ExternalInput").ap(),
        "madd": nc.dram_tensor("madd", [NPAIRS, P], bf16, kind="ExternalInput").ap(),
        "wq": nc.dram_tensor("wq", [E, E], bf16, kind="ExternalInput").ap(),
        "wk": nc.dram_tensor("wk", [E, E], bf16, kind="ExternalInput").ap(),
        "wv": nc.dram_tensor("wv", [E, E], bf16, kind="ExternalInput").ap(),
        "wo": nc.dram_tensor("wo", [E, E], bf16, kind="ExternalInput").ap(),
        "w1g": nc.dram_tensor("w1g", [E, F], bf16, kind="ExternalInput").ap(),
        "w2": nc.dram_tensor("w2", [F, E], bf16, kind="ExternalInput").ap(),
        "vecs": nc.dram_tensor("vecs", [P, 4 * EC], f32, kind="ExternalInput").ap(),
        "out": nc.dram_tensor("out", [P, EC * ntok], bf16, kind="ExternalOutput").ap(),
    }
    if has_g1:
        io["dg1"] = nc.dram_tensor(
            "dg1", [P, EC * P], bf16, kind="ExternalInput"
        ).ap()
    if has_t1:
        io["t1"] = nc.dram_tensor("t1", [1, F], f32, kind="ExternalInput").ap()
    if has_bsum:
        io["bsr"] = nc.dram_tensor("bsr", [1, E], f32, kind="ExternalInput").ap()
    with tile.TileContext(nc) as tc:
        for _ in range(reps):
            _emit(tc, io, has_t1, has_bsum, has_g2b, needs_max, has_g1)
    nc.compile()
    return nc


def _prep(value, key, query, mask, Wv, Wk, Wq, Wo, bo, W1, b1, W2, b2,
          g1, be1, g2, be2):
    """Returns (flags, per_core_maps)."""
    bfl = ml_dtypes.bfloat16
    f32n = np.float32
    g1 = np.asarray(g1, f32n)
    t1 = np.asarray(be1, f32n) @ np.asarray(W1, f32n) + np.asarray(b1, f32n)
    bsum = np.asarray(be1, f32n) + np.asarray(b2, f32n)
    has_t1 = bool(np.any(t1 != 0))
    has_bsum = bool(np.any(bsum != 0))
    has_g2b = bool(np.any(np.asarray(g2, f32n) != 1.0)
                   or np.any(np.asarray(be2, f32n) != 0.0))
    has_g1 = bool(np.any(g1 != 1.0))
    # stabilized softmax only needed if some query row is fully masked
    # (otherwise logits are O(5) and raw exp is safe)
    needs_max = bool(np.any(np.all(np.asarray(mask)[:, :, :, 0] == 0, axis=2)))

    w1g = (np.asarray(W1, f32n) * g1[:, None]).astype(bfl)
    dg1 = np.zeros((P, EC * P), f32n)
    idx = np.arange(P)
    for c in range(EC):
        dg1[idx, c * P + idx] = g1[c * P + idx]

    def cols(v):
        return np.asarray(v, f32n).reshape(EC, P).T  # [P, EC]

    vecs = np.concatenate(
        [cols(bo), cols(bsum), cols(g2), cols(be2)], axis=1
    ).astype(f32n)

    shared = {
        "wq": np.ascontiguousarray(np.asarray(Wq, f32n).astype(bfl)),
        "wk": np.ascontiguousarray(np.asarray(Wk, f32n).astype(bfl)),
        "wv": np.ascontiguousarray(np.asarray(Wv, f32n).astype(bfl)),
        "wo": np.ascontiguousarray(np.asarray(Wo, f32n).astype(bfl)),
        "w1g": np.ascontiguousarray(w1g),
        "w2": np.ascontiguousarray(np.asarray(W2, f32n).astype(bfl)),
        "vecs": np.ascontiguousarray(vecs),
    }
    if has_g1:
        shared["dg1"] = np.ascontiguousarray(dg1.astype(bfl))
    if has_t1:
        shared["t1"] = np.ascontiguousarray(t1.reshape(1, F).astype(bfl))
    if has_bsum:
        shared["bsr"] = np.ascontiguousarray(bsum.reshape(1, E).astype(bfl))

    def emaj(x2d):
        # [ntok, E] f32 -> [P, NG, EC, TOKG] bf16 -> flat [P, EC*ntok]
        a = np.asarray(x2d, f32n).reshape(NG, TOKG, EC, P)
        return np.ascontiguousarray(
            a.transpose(3, 0, 2, 1).astype(bfl).reshape(P, EC * NG * TOKG)
        )

    npc = 64 // NCORES
    ntok = NPAIRS * P
    in_maps = []
    for c in range(NCORES):
        nsl = slice(c * npc, (c + 1) * npc)
        madd = np.where(
            np.asarray(mask)[nsl, :, :, 0] == 0, f32n(-1e20), f32n(0.0)
        ).reshape(NPAIRS, P).astype(bfl)
        in_maps.append(
            {
                "xqt": emaj(np.asarray(query)[nsl].reshape(ntok, E)),
                "xkt": emaj(np.asarray(key)[nsl].reshape(ntok, E)),
                "xvt": emaj(np.asarray(value)[nsl].reshape(ntok, E)),
                "madd": np.ascontiguousarray(madd),
                **shared,
            }
        )
    return (has_t1, has_bsum, has_g2b, needs_max, has_g1), in_maps


def _prep_in_maps(**inputs):
    flags, in_maps = _prep(**{k: np.asarray(v) for k, v in inputs.items()})
    return in_maps


def _prep_flags(**inputs):
    flags, in_maps = _prep(**{k: np.asarray(v) for k, v in inputs.items()})
    return flags


def _gather(res):
    outs = []
    for r in res.results:
        a = r["out"].reshape(P, NG, EC, TOKG)
        outs.append(a.transpose(1, 3, 2, 0).reshape(NPAIRS * P, E))
    out = np.concatenate(outs, axis=0)
    return out.reshape(64, 2, P, E).astype(np.float32)


def gather_concat(out_concat):
    """Reassemble a [NCORES*P, EC*ntok] concatenated raw output (as produced
    by per-core sharding along axis 0) into the full [64, 2, P, E] result."""
    a = np.asarray(out_concat).reshape(NCORES, P, NG, EC, TOKG)
    a = a.transpose(0, 2, 4, 3, 1).reshape(NCORES * NPAIRS * P, E)
    return a.reshape(64, 2, P, E).astype(np.float32)


def kernel(**inputs) -> np.ndarray:
    flags, in_maps = _prep(**{k: np.asarray(v) for k, v in inputs.items()})
    nc = _build(*flags)
    res = run_bass_kernel_spmd(nc, in_maps, core_ids=list(range(NCORES)))
    return _gather(res)
